# revision 3
# baseline (speedup 1.0000x reference)
"""Distributed Trainium2 Bass kernel for GQA attention (B=2, S=2048, H=2048,
NH=16, NKV=4, HD=128) across 8 NeuronCores.

Sharding: core c -> (batch b = c//4, kv-group g = c%4).  Each core computes
Q/K/V projections for its 4 query heads + 1 kv head (column-sharded Wq/Wkv),
RoPE, causal flash-style attention in transposed layout (S^T = K Q^T so the
PV contraction lands on partitions), then AllGathers the per-group attention
outputs O^T across the 4 cores of its batch and applies a column shard of Wo
(full contraction, no all-reduce needed).  Output per core: y[b][:, 512g:512(g+1)].

Scheduling: the PE queue is kept dense by interleaving attention head-pairs
per k-chunk (the other head's score matmul hides the exp latency) and by
draining "fill" work (next block's QKV projections, previous block's Wo
contraction) between attention chunks via generators.  This keeps the tensor
engine p-state at max and hides ACT/DVE latency.

All matmul operands are bf16 (1 cycle/row on PE); accumulation is f32 in PSUM;
softmax runs without max-subtraction (scores are ~N(0,1), exp is safe in f32).
Causal structure is exploited at column granularity: for a diagonal k-chunk at
offset d, only score columns >= d are computed/exp'd/accumulated, and the
staircase boundary is handled by one [128,128] triangle multiply adding -40
on masked slots (exp then yields ~0 with no vector-engine mask op).
"""

import math
import sys

sys.path.insert(0, "/opt/trn_rl_repo")

import numpy as np
import ml_dtypes

import concourse.bass as bass
import concourse.mybir as mybir
import concourse.tile as tile
from concourse import bacc
from concourse import bass_utils
from concourse.bass import ds, ts

BF16 = mybir.dt.bfloat16
F32 = mybir.dt.float32
AF = mybir.ActivationFunctionType

HD = 128      # head dim
GQ = 4        # query heads per core
QC = GQ * HD  # query columns per core (512)
SB = 512      # sequence block
P = 128


def build_kernel(S=2048, H=2048):
    NB = S // SB          # number of seq blocks
    HO = H // P           # hidden contraction chunks
    ST = SB // P          # seq tiles per block (4)
    OC = H // 4           # output columns per core
    NPAIR = 2             # head pairs per core (AG granularity)

    nc = bacc.Bacc("TRN2", target_bir_lowering=False, debug=False, num_devices=8)

    xt = nc.dram_tensor("xt", [H, S], BF16, kind="ExternalInput").ap()
    wq = nc.dram_tensor("wq", [H, QC], BF16, kind="ExternalInput").ap()
    wk = nc.dram_tensor("wk", [H, HD], BF16, kind="ExternalInput").ap()
    wv = nc.dram_tensor("wv", [H, HD], BF16, kind="ExternalInput").ap()
    wo = nc.dram_tensor("wo", [2048, OC], BF16, kind="ExternalInput").ap()
    cost = nc.dram_tensor("cost", [HD, S], F32, kind="ExternalInput").ap()
    sint = nc.dram_tensor("sint", [HD, S], F32, kind="ExternalInput").ap()
    rotm = nc.dram_tensor("rotm", [HD, HD], BF16, kind="ExternalInput").ap()
    ident = nc.dram_tensor("ident", [HD, HD], BF16, kind="ExternalInput").ap()
    trineg = nc.dram_tensor("trineg", [HD, HD], BF16, kind="ExternalInput").ap()
    out = nc.dram_tensor("out", [S, OC], F32, kind="ExternalOutput").ap()

    xt_r = xt.rearrange("(ho p) s -> p ho s", p=P)
    wq_r = wq.rearrange("(ho p) m -> p ho m", p=P)
    wk_r = wk.rearrange("(ho p) m -> p ho m", p=P)
    wv_r = wv.rearrange("(ho p) m -> p ho m", p=P)
    wo_r = wo.rearrange("(oc p) n -> p oc n", p=P)

    with tile.TileContext(nc) as tc:
        with (
            tc.tile_pool(name="consts", bufs=1) as consts,
            tc.tile_pool(name="wpool", bufs=1) as wpool,
            tc.tile_pool(name="xtp", bufs=2) as xtp,
            tc.tile_pool(name="kvp", bufs=1) as kvp,
            tc.tile_pool(name="qfp", bufs=2) as qfp,
            tc.tile_pool(name="work", bufs=3) as work,
            tc.tile_pool(name="ptp", bufs=4) as ptp,
            tc.tile_pool(name="gp", bufs=2) as gp,
            tc.tile_pool(name="psQ", bufs=2, space="PSUM") as psQ,
            tc.tile_pool(name="psS", bufs=3, space="PSUM") as psS,
            tc.tile_pool(name="psO", bufs=2, space="PSUM") as psO,
            tc.tile_pool(name="psY", bufs=1, space="PSUM") as psY,
            tc.tile_pool(name="dram", bufs=1, space="DRAM") as dpool,
        ):
            # ---- critical-path prologue: exactly what qkv(0)+attn(0) need,
            # in consumption order; everything else is deferred ----
            xt_tiles = {}
            xt0 = xtp.tile([P, HO, SB], BF16, name="xt_sb")
            wq_chunks = []
            for ho in range(HO):
                wq_c = wpool.tile([P, QC], BF16, name=f"wq_c{ho}")
                nc.sync.dma_start(wq_c[:], wq_r[:, ho, :])
                nc.sync.dma_start(xt0[:, ho, :], xt_r[:, ho, ts(0, SB)])
                wq_chunks.append(wq_c)
            xt_tiles[0] = xt0
            wk_sb = wpool.tile([P, HO, HD], BF16, name="wk_sb")
            wv_sb = wpool.tile([P, HO, HD], BF16, name="wv_sb")
            for ho in range(HO):
                nc.sync.dma_start(wk_sb[:, ho, :], wk_r[:, ho, :])
                nc.sync.dma_start(wv_sb[:, ho, :], wv_r[:, ho, :])
            rotm_sb = consts.tile([P, HD], BF16, name="rotm_sb")
            nc.sync.dma_start(rotm_sb[:], rotm[:])
            cos_tiles, sin_tiles = [], []
            for j in range(NB):
                cos_tiles.append(consts.tile([P, SB], F32, name=f"cos_{j}"))
                sin_tiles.append(consts.tile([P, SB], F32, name=f"sin_{j}"))
            nc.sync.dma_start(cos_tiles[0][:], cost[:, ts(0, SB)])
            nc.sync.dma_start(sin_tiles[0][:], sint[:, ts(0, SB)])
            ident_sb = consts.tile([P, HD], BF16, name="ident_sb")
            nc.sync.dma_start(ident_sb[:], ident[:])
            trineg_sb = consts.tile([P, HD], BF16, name="trineg_sb")
            nc.sync.dma_start(trineg_sb[:], trineg[:])
            ones_sb = consts.tile([P, 1], BF16, name="ones_sb")
            nc.vector.memset(ones_sb[:], 1.0)

            # K^T and V for the whole sequence (grow per block)
            kT_sb = kvp.tile([P, S], BF16, name="kT_sb")   # [hd, s]
            v_sb = kvp.tile([P, S], BF16, name="v_sb")     # [s%128, kc*128+hd]

            ag_ins = [[None] * NPAIR for _ in range(NB)]
            ag_outs = [[None] * NPAIR for _ in range(NB)]
            for j in range(NB):
                for pr in range(NPAIR):
                    ag_ins[j][pr] = dpool.tile(
                        [2 * P, SB], BF16, name=f"ag_in_{j}_{pr}")
                    ag_outs[j][pr] = dpool.tile(
                        [8 * P, SB], BF16, name=f"ag_out_{j}_{pr}")

            def rope(out_ap, ps_raw, j):
                """out = ps_raw*cos + (rot @ ps_raw)*sin, written as bf16."""
                q_raw = work.tile([P, SB], BF16, tag="qraw", name="q_raw")
                nc.vector.tensor_copy(q_raw[:], ps_raw[:])
                ps_rot = psQ.tile([P, SB], F32, tag="ps", name="ps_rot")
                nc.tensor.matmul(ps_rot[:], rotm_sb[:], q_raw[:], start=True, stop=True)
                t1 = work.tile([P, SB], F32, tag="t1", name="t1", bufs=2)
                nc.vector.tensor_mul(t1[:], ps_raw[:], cos_tiles[j][:])
                t2 = work.tile([P, SB], F32, tag="t2", name="t2", bufs=2)
                nc.vector.tensor_mul(t2[:], ps_rot[:], sin_tiles[j][:])
                nc.vector.tensor_add(out_ap, t1[:], t2[:])

            q_map = {}

            def qkv_steps(j):
                """Generator: projection work for block j in ~0.9us PE steps."""
                xt_sb = xt_tiles[j]
                # K head
                ps_k = psQ.tile([P, SB], F32, tag="ps", name="ps_k")
                for g4 in range(4):
                    for ho in range(4 * g4, 4 * g4 + 4):
                        nc.tensor.matmul(
                            ps_k[:], wk_sb[:, ho, :], xt_sb[:, ho, :],
                            start=(ho == 0), stop=(ho == HO - 1),
                        )
                    yield
                rope(kT_sb[:, ts(j, SB)], ps_k, j)
                yield
                # V head (transposed layout: [s%128, hd])
                ps_v = psQ.tile([P, SB], F32, tag="ps", name="ps_v")
                for st in range(ST):
                    for ho in range(HO):
                        nc.tensor.matmul(
                            ps_v[:, ts(st, P)], xt_sb[:, ho, ts(st, P)], wv_sb[:, ho, :],
                            start=(ho == 0), stop=(ho == HO - 1),
                        )
                    yield
                nc.vector.tensor_copy(v_sb[:, ts(j, SB)], ps_v[:])
                yield
                # Q heads
                q_all = qfp.tile([P, GQ, SB], BF16, name="q_all")
                q_map[j] = q_all
                for qc in range(GQ):
                    ps_q = psQ.tile([P, SB], F32, tag="ps", name="ps_q")
                    for g4 in range(4):
                        for ho in range(4 * g4, 4 * g4 + 4):
                            nc.tensor.matmul(
                                ps_q[:], wq_chunks[ho][:, ts(qc, P)], xt_sb[:, ho, :],
                                start=(ho == 0), stop=(ho == HO - 1),
                            )
                        yield
                    rope(q_all[:, qc, :], ps_q, j)
                    yield

            g_loaded = {}

            def g_load(j, pr):
                """Load the gathered O^T chunks of AG(j, pr) into SBUF."""
                ag_r = ag_outs[j][pr].rearrange("(c p) s -> p c s", p=P)
                g_cs = []
                for c in range(8):
                    g_c = gp.tile([P, SB], BF16, tag=f"g{c}", name=f"g_c{c}")
                    nc.sync.dma_start(g_c[:], ag_r[:, c, :])
                    g_cs.append(g_c)
                g_loaded[(j, pr)] = g_cs

            y_parts = {}

            def wo_steps(j, pr):
                """Generator: Wo contraction for (block j, pair pr)."""
                g_cs = g_loaded.pop((j, pr))
                for st in range(ST):
                    ps_y = psY.tile([P, OC], F32, tag="psy", name="ps_y")
                    for c in range(8):
                        r, q = c // 2, c % 2
                        ocg = 4 * r + 2 * pr + q
                        nc.tensor.matmul(
                            ps_y[:], g_cs[c][:, ts(st, P)], wo_sb[:, ocg, :],
                            start=(c == 0), stop=(c == 7),
                        )
                        if c == 3:
                            yield
                    if pr == 0:
                        yp = work.tile([P, OC], F32, tag="ypart",
                                       name="y_part", bufs=5)
                        nc.vector.tensor_copy(yp[:], ps_y[:])
                        y_parts[(j, st)] = yp
                    else:
                        y_sb = work.tile([P, OC], F32, tag="ysb", name="y_sb", bufs=2)
                        nc.vector.tensor_add(
                            y_sb[:], y_parts.pop((j, st))[:], ps_y[:])
                        nc.sync.dma_start(
                            out[ds(j * SB + st * P, P), :], y_sb[:])
                    yield

            # ---- fill machinery: drain pending generators one step at a time
            fill_q = []

            def emit_fill(n):
                done = 0
                while fill_q and done < n:
                    try:
                        next(fill_q[0])
                        done += 1
                    except StopIteration:
                        fill_q.pop(0)

            def drain_fill():
                while fill_q:
                    try:
                        next(fill_q[0])
                    except StopIteration:
                        fill_q.pop(0)

            def attn_sweep(j, pr):
                """Causal attention for heads (2pr, 2pr+1) of query block j,
                with fill work interleaved between k-chunks."""
                h0, h1 = 2 * pr, 2 * pr + 1
                q_all = q_map[j]
                KC = 4 * (j + 1)
                ps_o = {}
                accs = {}
                for h in (h0, h1):
                    ps_o[h] = psO.tile([P, SB], F32, tag="pso", name="ps_o")
                    accs[h] = [
                        work.tile([P, SB], BF16, tag=f"acc{h % 2}a", name="acc_a"),
                        work.tile([P, SB], BF16, tag=f"acc{h % 2}b", name="acc_b"),
                    ]
                pts = {}
                for kc in range(KC):
                    diag = kc >= 4 * j
                    d = P * (kc - 4 * j) if diag else 0
                    for h in (h0, h1):
                        ps_s = psS.tile([P, SB], F32, tag="pss", name="ps_s")
                        nc.tensor.matmul(
                            ps_s[:, d:], kT_sb[:, ts(kc, P)], q_all[:, h, d:],
                            start=True, stop=not diag,
                        )
                        if diag:
                            nc.tensor.matmul(
                                ps_s[:, d:d + P], ident_sb[:], trineg_sb[:],
                                start=False, stop=True,
                            )
                        pt = ptp.tile([P, SB], BF16, tag=f"pt{h % 2}", name="pt")
                        nc.scalar.activation(pt[:, d:], ps_s[:, d:], AF.Exp)
                        pts[h] = pt
                    emit_fill(1)
                    for h in (h0, h1):
                        pt = pts[h]
                        acc = accs[h][kc % 2]
                        if kc < 2:
                            if d > 0:
                                nc.vector.memset(acc[:, :d], 0.0)
                            nc.vector.tensor_copy(acc[:, d:], pt[:, d:])
                        else:
                            nc.vector.tensor_add(acc[:, d:], acc[:, d:], pt[:, d:])
                        nc.tensor.matmul(
                            ps_o[h][:, d:], v_sb[:, ts(kc, P)], pt[:, d:],
                            start=(kc == 0), stop=(kc == KC - 1),
                        )
                for h in (h0, h1):
                    ps_d = psS.tile([1, SB], F32, tag="pss", name="ps_d")
                    nc.tensor.matmul(ps_d[:], ones_sb[:], accs[h][0][:],
                                     start=True, stop=False)
                    nc.tensor.matmul(ps_d[:], ones_sb[:], accs[h][1][:],
                                     start=False, stop=True)
                    recip = work.tile([1, SB], F32, tag="recip", name="recip")
                    nc.vector.reciprocal_approx_fast(recip[:], ps_d[:])
                    rb = work.tile([P, SB], F32, tag="rb", name="rb", bufs=2)
                    nc.gpsimd.partition_broadcast(rb[:], recip[:], channels=P)
                    o_sb = work.tile([P, SB], BF16, tag="osb", name="o_sb")
                    nc.vector.tensor_mul(o_sb[:], ps_o[h][:], rb[:])
                    nc.sync.dma_start(ag_ins[j][pr][ts(h % 2, P), :], o_sb[:])
                    emit_fill(1)
                nc.gpsimd.collective_compute(
                    "AllGather", mybir.AluOpType.bypass,
                    replica_groups=[[0, 1, 2, 3], [4, 5, 6, 7]],
                    ins=[ag_ins[j][pr][:].opt()],
                    outs=[ag_outs[j][pr][:].opt()],
                )

            # ---- emission schedule ----
            # qkv(0) runs un-filled (DMA-gated startup)
            for _ in qkv_steps(0):
                pass

            # deferred prologue: needed from iter 0's fill onward
            for j in range(1, NB):
                nc.sync.dma_start(cos_tiles[j][:], cost[:, ts(j, SB)])
                nc.sync.dma_start(sin_tiles[j][:], sint[:, ts(j, SB)])
            xt1 = xtp.tile([P, HO, SB], BF16, name="xt_sb")
            for ho in range(HO):
                nc.sync.dma_start(xt1[:, ho, :], xt_r[:, ho, ts(1, SB)])
            xt_tiles[1] = xt1
            wo_sb = wpool.tile([P, 16, OC], BF16, name="wo_sb")
            for oc in range(16):
                nc.sync.dma_start(wo_sb[:, oc, :], wo_r[:, oc, :])

            for j in range(NB):
                # fill sources for this iteration
                if j + 1 < NB:
                    fill_q.append(qkv_steps(j + 1))
                if j == NB - 1 and j >= 1:
                    # last iter has no qkv fill: use wo(j-1) inside sweeps
                    g_load(j - 1, 0)
                    fill_q.append(wo_steps(j - 1, 0))
                attn_sweep(j, 0)
                if j >= 1 and j < NB - 1:
                    g_load(j - 1, 0)
                    g_load(j - 1, 1)
                if j == NB - 1 and j >= 1:
                    g_load(j - 1, 1)
                    fill_q.append(wo_steps(j - 1, 1))
                attn_sweep(j, 1)
                if j + 2 < NB:
                    xt_n = xtp.tile([P, HO, SB], BF16, name="xt_sb")
                    for ho in range(HO):
                        nc.sync.dma_start(xt_n[:, ho, :], xt_r[:, ho, ts(j + 2, SB)])
                    xt_tiles[j + 2] = xt_n
                if j >= 1 and j < NB - 1:
                    fill_q.append(wo_steps(j - 1, 0))
                    fill_q.append(wo_steps(j - 1, 1))
                drain_fill()

            # tail: wo for the last block
            g_load(NB - 1, 0)
            for _ in wo_steps(NB - 1, 0):
                pass
            g_load(NB - 1, 1)
            for _ in wo_steps(NB - 1, 1):
                pass

    return nc


def make_in_maps(x, cos, sin, Wq, Wkv, Wo, S=2048, H=2048):
    bf = ml_dtypes.bfloat16
    scale = 1.0 / math.sqrt(HD)
    NKVH = Wkv.shape[1] // (2 * HD)  # 4
    OC = H // 4

    Prot = np.zeros((HD, HD), np.float32)
    Prot[np.arange(64), np.arange(64) + 64] = -1.0
    Prot[np.arange(64) + 64, np.arange(64)] = 1.0
    rotm = np.ascontiguousarray(Prot.T).astype(bf)

    kk = np.arange(P)[:, None]
    w = np.arange(HD)[None, :]
    trineg_np = np.where(w < kk, -40.0, 0.0).astype(np.float32).astype(bf)
    ident_np = np.eye(HD, dtype=np.float32).astype(bf)

    cost = np.ascontiguousarray(cos.T).astype(np.float32)
    sint = np.ascontiguousarray(sin.T).astype(np.float32)

    in_maps = []
    for c in range(8):
        b, g = c // 4, c % 4
        in_maps.append({
            "xt": np.ascontiguousarray(np.asarray(x)[b].T).astype(bf),
            "wq": np.ascontiguousarray(np.asarray(Wq)[:, QC * g:QC * (g + 1)] * scale).astype(bf),
            "wk": np.ascontiguousarray(np.asarray(Wkv)[:, HD * g:HD * (g + 1)]).astype(bf),
            "wv": np.ascontiguousarray(
                np.asarray(Wkv)[:, NKVH * HD + HD * g:NKVH * HD + HD * (g + 1)]).astype(bf),
            "wo": np.ascontiguousarray(np.asarray(Wo)[:, OC * g:OC * (g + 1)]).astype(bf),
            "cost": cost, "sint": sint, "rotm": rotm,
            "ident": ident_np, "trineg": trineg_np,
        })
    return in_maps


_CACHE = {}


def _get_nc(S=2048, H=2048):
    key = (S, H)
    if key not in _CACHE:
        nc = build_kernel(S, H)
        nc.compile()
        _CACHE[key] = nc
    return _CACHE[key]


def run(x, cos, sin, Wq, Wkv, Wo, trace=False):
    S, H = 2048, 2048
    nc = _get_nc(S, H)
    in_maps = make_in_maps(x, cos, sin, Wq, Wkv, Wo, S, H)
    res = bass_utils.run_bass_kernel_spmd(
        nc, in_maps, core_ids=list(range(8)), trace=trace
    )
    OC = H // 4
    y = np.empty((2, S, H), np.float32)
    for c in range(8):
        b, g = c // 4, c % 4
        y[b][:, OC * g:OC * (g + 1)] = res.results[c]["out"]
    return y, res


def kernel(x, cos, sin, Wq, Wkv, Wo):
    y, _ = run(x, cos, sin, Wq, Wkv, Wo, trace=False)
    return y


# revision 5
# speedup vs baseline: 1.0385x; 1.0385x over previous
"""Distributed Trainium2 Bass kernel for GQA attention (B=2, S=2048, H=2048,
NH=16, NKV=4, HD=128) across 8 NeuronCores.

Sharding: core c -> (batch b = c//4, kv-group g = c%4).  Each core computes
Q/K/V projections for its 4 query heads + 1 kv head (column-sharded Wq/Wkv),
RoPE, causal flash-style attention in transposed layout (S^T = K Q^T so the
PV contraction lands on partitions), then AllGathers the per-group attention
outputs O^T across the 4 cores of its batch and applies a column shard of Wo
(full contraction, no all-reduce needed).  Output per core: y[b][:, 512g:512(g+1)].

Scheduling: the PE queue is kept dense by interleaving attention head-pairs
per k-chunk (the other head's score matmul hides the exp latency) and by
draining "fill" work (next block's QKV projections, previous block's Wo
contraction) between attention chunks via generators.  This keeps the tensor
engine p-state at max and hides ACT/DVE latency.

All matmul operands are bf16 (1 cycle/row on PE); accumulation is f32 in PSUM;
softmax runs without max-subtraction (scores are ~N(0,1), exp is safe in f32).
Causal structure is exploited at column granularity: for a diagonal k-chunk at
offset d, only score columns >= d are computed/exp'd/accumulated, and the
staircase boundary is handled by one [128,128] triangle multiply adding -40
on masked slots (exp then yields ~0 with no vector-engine mask op).
"""

import math
import sys

sys.path.insert(0, "/opt/trn_rl_repo")

import numpy as np
import ml_dtypes

import concourse.bass as bass
import concourse.mybir as mybir
import concourse.tile as tile
from concourse import bacc
from concourse import bass_utils
from concourse.bass import ds, ts

BF16 = mybir.dt.bfloat16
F32 = mybir.dt.float32
AF = mybir.ActivationFunctionType

HD = 128      # head dim
GQ = 4        # query heads per core
QC = GQ * HD  # query columns per core (512)
SB = 512      # sequence block
P = 128


def build_kernel(S=2048, H=2048):
    NB = S // SB          # number of seq blocks
    HO = H // P           # hidden contraction chunks
    ST = SB // P          # seq tiles per block (4)
    OC = H // 4           # output columns per core
    NPAIR = 2             # head pairs per core (AG granularity)

    nc = bacc.Bacc("TRN2", target_bir_lowering=False, debug=False, num_devices=8)

    xt = nc.dram_tensor("xt", [H, S], BF16, kind="ExternalInput").ap()
    wq = nc.dram_tensor("wq", [H, QC], BF16, kind="ExternalInput").ap()
    wk = nc.dram_tensor("wk", [H, HD], BF16, kind="ExternalInput").ap()
    wv = nc.dram_tensor("wv", [H, HD], BF16, kind="ExternalInput").ap()
    wo = nc.dram_tensor("wo", [2048, OC], BF16, kind="ExternalInput").ap()
    cost = nc.dram_tensor("cost", [HD, S], F32, kind="ExternalInput").ap()
    sint = nc.dram_tensor("sint", [HD, S], F32, kind="ExternalInput").ap()
    rotm = nc.dram_tensor("rotm", [HD, HD], BF16, kind="ExternalInput").ap()
    ident = nc.dram_tensor("ident", [HD, HD], BF16, kind="ExternalInput").ap()
    trineg = nc.dram_tensor("trineg", [HD, HD], BF16, kind="ExternalInput").ap()
    out = nc.dram_tensor("out", [S, OC], F32, kind="ExternalOutput").ap()

    xt_r = xt.rearrange("(ho p) s -> p ho s", p=P)
    wq_r = wq.rearrange("(ho p) m -> p ho m", p=P)
    wk_r = wk.rearrange("(ho p) m -> p ho m", p=P)
    wv_r = wv.rearrange("(ho p) m -> p ho m", p=P)
    wo_r = wo.rearrange("(oc p) n -> p oc n", p=P)

    with tile.TileContext(nc) as tc:
        with (
            tc.tile_pool(name="consts", bufs=1) as consts,
            tc.tile_pool(name="wpool", bufs=1) as wpool,
            tc.tile_pool(name="xtp", bufs=2) as xtp,
            tc.tile_pool(name="kvp", bufs=1) as kvp,
            tc.tile_pool(name="qfp", bufs=2) as qfp,
            tc.tile_pool(name="work", bufs=3) as work,
            tc.tile_pool(name="ptp", bufs=4) as ptp,
            tc.tile_pool(name="gp", bufs=2) as gp,
            tc.tile_pool(name="psQ", bufs=2, space="PSUM") as psQ,
            tc.tile_pool(name="psS", bufs=3, space="PSUM") as psS,
            tc.tile_pool(name="psO", bufs=2, space="PSUM") as psO,
            tc.tile_pool(name="psY", bufs=1, space="PSUM") as psY,
            tc.tile_pool(name="dram", bufs=1, space="DRAM") as dpool,
        ):
            # ---- critical-path prologue: K-projection inputs first so the
            # PE starts within a few us; per-head Wq tiles so attention can
            # begin after just heads 0/1 arrive; everything else deferred ----
            xt_tiles = {}
            xt0 = xtp.tile([P, HO, SB], BF16, name="xt_sb")
            wk_sb = wpool.tile([P, HO, HD], BF16, name="wk_sb")
            wv_sb = wpool.tile([P, HO, HD], BF16, name="wv_sb")
            for ho in range(HO):
                nc.sync.dma_start(wk_sb[:, ho, :], wk_r[:, ho, :])
                nc.sync.dma_start(xt0[:, ho, :], xt_r[:, ho, ts(0, SB)])
            xt_tiles[0] = xt0
            for ho in range(HO):
                nc.sync.dma_start(wv_sb[:, ho, :], wv_r[:, ho, :])
            rotm_sb = consts.tile([P, HD], BF16, name="rotm_sb")
            nc.sync.dma_start(rotm_sb[:], rotm[:])
            cos_tiles, sin_tiles = [], []
            for j in range(NB):
                cos_tiles.append(consts.tile([P, SB], F32, name=f"cos_{j}"))
                sin_tiles.append(consts.tile([P, SB], F32, name=f"sin_{j}"))
            nc.sync.dma_start(cos_tiles[0][:], cost[:, ts(0, SB)])
            nc.sync.dma_start(sin_tiles[0][:], sint[:, ts(0, SB)])
            ident_sb = consts.tile([P, HD], BF16, name="ident_sb")
            nc.sync.dma_start(ident_sb[:], ident[:])
            trineg_sb = consts.tile([P, HD], BF16, name="trineg_sb")
            nc.sync.dma_start(trineg_sb[:], trineg[:])
            ones_sb = consts.tile([P, 1], BF16, name="ones_sb")
            nc.vector.memset(ones_sb[:], 1.0)
            wqh = []
            for qc in range(GQ):
                wqh.append(wpool.tile([P, HO, HD], BF16, name=f"wqh{qc}"))
            for qc in range(GQ):
                for ho in range(HO):
                    nc.sync.dma_start(wqh[qc][:, ho, :], wq_r[:, ho, ts(qc, P)])
                if qc == 1:
                    break
            # (wqh2/3, xt1, cos/sin 1-3, wo are emitted after qkv(0) below)

            # K^T and V for the whole sequence (grow per block)
            kT_sb = kvp.tile([P, S], BF16, name="kT_sb")   # [hd, s]
            v_sb = kvp.tile([P, S], BF16, name="v_sb")     # [s%128, kc*128+hd]

            ag_ins = [[None] * NPAIR for _ in range(NB)]
            ag_outs = [[None] * NPAIR for _ in range(NB)]
            for j in range(NB):
                for pr in range(NPAIR):
                    ag_ins[j][pr] = dpool.tile(
                        [2 * P, SB], BF16, name=f"ag_in_{j}_{pr}")
                    ag_outs[j][pr] = dpool.tile(
                        [8 * P, SB], BF16, name=f"ag_out_{j}_{pr}")

            # ---- split rope: the PSUM->SBUF copy (DVE) is emitted with the
            # projection; the rotation matmul + muls run >=1 fill step later
            # so the PE never waits on the copy ----
            rope_pend = []

            def flush_rope():
                while rope_pend:
                    rope_pend.pop(0)()

            def sched_rope(out_ap, ps_raw, j):
                q_raw = work.tile([P, SB], BF16, tag="qraw", name="q_raw")
                nc.vector.tensor_copy(q_raw[:], ps_raw[:])

                def rope_b():
                    ps_rot = psS.tile([P, SB], F32, tag="pss", name="ps_rot")
                    nc.tensor.matmul(ps_rot[:], rotm_sb[:], q_raw[:],
                                     start=True, stop=True)
                    t1 = work.tile([P, SB], F32, tag="t1", name="t1", bufs=2)
                    nc.vector.tensor_mul(t1[:], ps_raw[:], cos_tiles[j][:])
                    t2 = work.tile([P, SB], F32, tag="t2", name="t2", bufs=2)
                    nc.vector.tensor_mul(t2[:], ps_rot[:], sin_tiles[j][:])
                    nc.vector.tensor_add(out_ap, t1[:], t2[:])
                rope_pend.append(rope_b)

            q_map = {}

            def kv_steps(j):
                """Generator: K/V projection for block j in ~0.9us PE steps."""
                xt_sb = xt_tiles[j]
                ps_k = psQ.tile([P, SB], F32, tag="ps", name="ps_k")
                for g4 in range(4):
                    for ho in range(4 * g4, 4 * g4 + 4):
                        nc.tensor.matmul(
                            ps_k[:], wk_sb[:, ho, :], xt_sb[:, ho, :],
                            start=(ho == 0), stop=(ho == HO - 1),
                        )
                    flush_rope()
                    yield
                sched_rope(kT_sb[:, ts(j, SB)], ps_k, j)
                yield
                ps_v = psQ.tile([P, SB], F32, tag="ps", name="ps_v")
                for st in range(ST):
                    for ho in range(HO):
                        nc.tensor.matmul(
                            ps_v[:, ts(st, P)], xt_sb[:, ho, ts(st, P)], wv_sb[:, ho, :],
                            start=(ho == 0), stop=(ho == HO - 1),
                        )
                    if st == 0:
                        flush_rope()
                    yield
                nc.vector.tensor_copy(v_sb[:, ts(j, SB)], ps_v[:])
                yield

            def q_steps(j, qcs):
                """Generator: Q-head projections + rope for block j."""
                xt_sb = xt_tiles[j]
                if j not in q_map:
                    q_map[j] = qfp.tile([P, GQ, SB], BF16, name="q_all")
                q_all = q_map[j]
                for qc in qcs:
                    ps_q = psQ.tile([P, SB], F32, tag="ps", name="ps_q")
                    for g4 in range(4):
                        for ho in range(4 * g4, 4 * g4 + 4):
                            nc.tensor.matmul(
                                ps_q[:], wqh[qc][:, ho, :], xt_sb[:, ho, :],
                                start=(ho == 0), stop=(ho == HO - 1),
                            )
                        if g4 == 1:
                            flush_rope()
                        yield
                    sched_rope(q_all[:, qc, :], ps_q, j)
                    yield
                yield
                flush_rope()

            g_loaded = {}

            def g_load(j, pr):
                """Load the gathered O^T chunks of AG(j, pr) into SBUF."""
                ag_r = ag_outs[j][pr].rearrange("(c p) s -> p c s", p=P)
                g_cs = []
                for c in range(8):
                    g_c = gp.tile([P, SB], BF16, tag=f"g{c}", name=f"g_c{c}")
                    nc.sync.dma_start(g_c[:], ag_r[:, c, :])
                    g_cs.append(g_c)
                g_loaded[(j, pr)] = g_cs

            y_parts = {}

            def wo_steps(j, pr):
                """Generator: Wo contraction for (block j, pair pr)."""
                g_cs = g_loaded.pop((j, pr))
                for st in range(ST):
                    ps_y = psY.tile([P, OC], F32, tag="psy", name="ps_y")
                    for c in range(8):
                        r, q = c // 2, c % 2
                        ocg = 4 * r + 2 * pr + q
                        nc.tensor.matmul(
                            ps_y[:], g_cs[c][:, ts(st, P)], wo_sb[:, ocg, :],
                            start=(c == 0), stop=(c == 7),
                        )
                        if c == 3:
                            yield
                    if pr == 0:
                        yp = work.tile([P, OC], F32, tag="ypart",
                                       name="y_part", bufs=5)
                        nc.vector.tensor_copy(yp[:], ps_y[:])
                        y_parts[(j, st)] = yp
                    else:
                        y_sb = work.tile([P, OC], F32, tag="ysb", name="y_sb",
                                         bufs=2)
                        nc.vector.tensor_add(
                            y_sb[:], y_parts.pop((j, st))[:], ps_y[:])
                        nc.scalar.dma_start(
                            out[ds(j * SB + st * P, P), :], y_sb[:])
                    yield

            # ---- fill machinery ----
            fill_q = []

            def emit_fill(n):
                done = 0
                while fill_q and done < n:
                    try:
                        next(fill_q[0])
                        done += 1
                    except StopIteration:
                        fill_q.pop(0)

            def drain_fill():
                while fill_q:
                    try:
                        next(fill_q[0])
                    except StopIteration:
                        fill_q.pop(0)

            def attn_sweep(j, pr):
                """Causal attention for heads (2pr, 2pr+1) of query block j,
                with fill work interleaved between k-chunks."""
                h0, h1 = 2 * pr, 2 * pr + 1
                q_all = q_map[j]
                KC = 4 * (j + 1)
                ps_o = {}
                accs = {}
                for h in (h0, h1):
                    ps_o[h] = psO.tile([P, SB], F32, tag="pso", name="ps_o")
                    accs[h] = [
                        work.tile([P, SB], BF16, tag=f"acc{h % 2}a",
                                  name="acc_a", bufs=2),
                        work.tile([P, SB], BF16, tag=f"acc{h % 2}b",
                                  name="acc_b", bufs=2),
                    ]
                pts = {}
                for kc in range(KC):
                    diag = kc >= 4 * j
                    d = P * (kc - 4 * j) if diag else 0
                    for h in (h0, h1):
                        ps_s = psS.tile([P, SB], F32, tag="pss", name="ps_s")
                        nc.tensor.matmul(
                            ps_s[:, d:], kT_sb[:, ts(kc, P)], q_all[:, h, d:],
                            start=True, stop=not diag,
                        )
                        if diag:
                            nc.tensor.matmul(
                                ps_s[:, d:d + P], ident_sb[:], trineg_sb[:],
                                start=False, stop=True,
                            )
                        pt = ptp.tile([P, SB], BF16, tag=f"pt{h % 2}", name="pt")
                        nc.scalar.activation(pt[:, d:], ps_s[:, d:], AF.Exp)
                        pts[h] = pt
                    emit_fill(1)
                    for h in (h0, h1):
                        pt = pts[h]
                        acc = accs[h][kc % 2]
                        if kc < 2:
                            if d > 0:
                                nc.vector.memset(acc[:, :d], 0.0)
                            nc.vector.tensor_copy(acc[:, d:], pt[:, d:])
                        else:
                            nc.vector.tensor_add(acc[:, d:], acc[:, d:], pt[:, d:])
                        nc.tensor.matmul(
                            ps_o[h][:, d:], v_sb[:, ts(kc, P)], pt[:, d:],
                            start=(kc == 0), stop=(kc == KC - 1),
                        )
                for h in (h0, h1):
                    ps_d = psS.tile([1, SB], F32, tag="pss", name="ps_d")
                    nc.tensor.matmul(ps_d[:], ones_sb[:], accs[h][0][:],
                                     start=True, stop=False)
                    nc.tensor.matmul(ps_d[:], ones_sb[:], accs[h][1][:],
                                     start=False, stop=True)
                    recip = work.tile([1, SB], F32, tag="recip", name="recip")
                    nc.vector.reciprocal_approx_fast(recip[:], ps_d[:])
                    rb = work.tile([P, SB], F32, tag="rb", name="rb", bufs=2)
                    nc.gpsimd.partition_broadcast(rb[:], recip[:], channels=P)
                    o_sb = work.tile([P, SB], BF16, tag="osb", name="o_sb")
                    nc.vector.tensor_mul(o_sb[:], ps_o[h][:], rb[:])
                    nc.scalar.dma_start(ag_ins[j][pr][ts(h % 2, P), :], o_sb[:])
                    emit_fill(1)
                nc.gpsimd.collective_compute(
                    "AllGather", mybir.AluOpType.bypass,
                    replica_groups=[[0, 1, 2, 3], [4, 5, 6, 7]],
                    ins=[ag_ins[j][pr][:].opt()],
                    outs=[ag_outs[j][pr][:].opt()],
                )

            # ---- emission schedule ----
            # qkv(0): K, V, heads 0/1 inline (DMA-gated startup); heads 2/3
            # become sweep fill so attention starts as soon as q1 is roped.
            for _ in kv_steps(0):
                pass
            for _ in q_steps(0, (0, 1)):
                pass
            q0_tail = q_steps(0, (2, 3))
            fill_q.append(q0_tail)

            # deferred prologue: needed from iter 0's fill onward
            for qc in (2, 3):
                for ho in range(HO):
                    nc.sync.dma_start(wqh[qc][:, ho, :], wq_r[:, ho, ts(qc, P)])
            xt1 = xtp.tile([P, HO, SB], BF16, name="xt_sb")
            for ho in range(HO):
                nc.sync.dma_start(xt1[:, ho, :], xt_r[:, ho, ts(1, SB)])
            xt_tiles[1] = xt1
            for j in range(1, NB):
                nc.sync.dma_start(cos_tiles[j][:], cost[:, ts(j, SB)])
                nc.sync.dma_start(sin_tiles[j][:], sint[:, ts(j, SB)])
            wo_sb = wpool.tile([P, 16, OC], BF16, name="wo_sb")
            for oc in range(16):
                nc.sync.dma_start(wo_sb[:, oc, :], wo_r[:, oc, :])

            for j in range(NB):
                if j + 1 < NB:
                    fill_q.append(kv_steps(j + 1))
                    fill_q.append(q_steps(j + 1, (0, 1, 2, 3)))
                if j == NB - 1 and j >= 1:
                    # last iter has no qkv fill: use wo(j-1) inside sweeps
                    g_load(j - 1, 0)
                    fill_q.append(wo_steps(j - 1, 0))
                attn_sweep(j, 0)
                if j == 0:
                    # heads 2/3 of block 0 must be projected+roped before
                    # sweep(0,1) reads them
                    while q0_tail in fill_q:
                        try:
                            next(q0_tail)
                        except StopIteration:
                            fill_q.remove(q0_tail)
                if j >= 1 and j < NB - 1:
                    g_load(j - 1, 0)
                    g_load(j - 1, 1)
                if j == NB - 1 and j >= 1:
                    g_load(j - 1, 1)
                    fill_q.append(wo_steps(j - 1, 1))
                attn_sweep(j, 1)
                if j + 2 < NB:
                    xt_n = xtp.tile([P, HO, SB], BF16, name="xt_sb")
                    for ho in range(HO):
                        nc.sync.dma_start(xt_n[:, ho, :], xt_r[:, ho, ts(j + 2, SB)])
                    xt_tiles[j + 2] = xt_n
                if j >= 1 and j < NB - 1:
                    fill_q.append(wo_steps(j - 1, 0))
                    fill_q.append(wo_steps(j - 1, 1))
                drain_fill()

            # tail: wo for the last block
            g_load(NB - 1, 0)
            for _ in wo_steps(NB - 1, 0):
                pass
            g_load(NB - 1, 1)
            for _ in wo_steps(NB - 1, 1):
                pass

    return nc


def make_in_maps(x, cos, sin, Wq, Wkv, Wo, S=2048, H=2048):
    bf = ml_dtypes.bfloat16
    scale = 1.0 / math.sqrt(HD)
    NKVH = Wkv.shape[1] // (2 * HD)  # 4
    OC = H // 4

    Prot = np.zeros((HD, HD), np.float32)
    Prot[np.arange(64), np.arange(64) + 64] = -1.0
    Prot[np.arange(64) + 64, np.arange(64)] = 1.0
    rotm = np.ascontiguousarray(Prot.T).astype(bf)

    kk = np.arange(P)[:, None]
    w = np.arange(HD)[None, :]
    trineg_np = np.where(w < kk, -40.0, 0.0).astype(np.float32).astype(bf)
    ident_np = np.eye(HD, dtype=np.float32).astype(bf)

    cost = np.ascontiguousarray(cos.T).astype(np.float32)
    sint = np.ascontiguousarray(sin.T).astype(np.float32)

    in_maps = []
    for c in range(8):
        b, g = c // 4, c % 4
        in_maps.append({
            "xt": np.ascontiguousarray(np.asarray(x)[b].T).astype(bf),
            "wq": np.ascontiguousarray(np.asarray(Wq)[:, QC * g:QC * (g + 1)] * scale).astype(bf),
            "wk": np.ascontiguousarray(np.asarray(Wkv)[:, HD * g:HD * (g + 1)]).astype(bf),
            "wv": np.ascontiguousarray(
                np.asarray(Wkv)[:, NKVH * HD + HD * g:NKVH * HD + HD * (g + 1)]).astype(bf),
            "wo": np.ascontiguousarray(np.asarray(Wo)[:, OC * g:OC * (g + 1)]).astype(bf),
            "cost": cost, "sint": sint, "rotm": rotm,
            "ident": ident_np, "trineg": trineg_np,
        })
    return in_maps


_CACHE = {}


def _get_nc(S=2048, H=2048):
    key = (S, H)
    if key not in _CACHE:
        nc = build_kernel(S, H)
        nc.compile()
        _CACHE[key] = nc
    return _CACHE[key]


def run(x, cos, sin, Wq, Wkv, Wo, trace=False):
    S, H = 2048, 2048
    nc = _get_nc(S, H)
    in_maps = make_in_maps(x, cos, sin, Wq, Wkv, Wo, S, H)
    res = bass_utils.run_bass_kernel_spmd(
        nc, in_maps, core_ids=list(range(8)), trace=trace
    )
    OC = H // 4
    y = np.empty((2, S, H), np.float32)
    for c in range(8):
        b, g = c // 4, c % 4
        y[b][:, OC * g:OC * (g + 1)] = res.results[c]["out"]
    return y, res


def kernel(x, cos, sin, Wq, Wkv, Wo):
    y, _ = run(x, cos, sin, Wq, Wkv, Wo, trace=False)
    return y


# revision 6
# speedup vs baseline: 1.0626x; 1.0233x over previous
"""Distributed Trainium2 Bass kernel for GQA attention (B=2, S=2048, H=2048,
NH=16, NKV=4, HD=128) across 8 NeuronCores.

Sharding: core c -> (batch b = c//4, kv-group g = c%4).  Each core computes
Q/K/V projections for its 4 query heads + 1 kv head (column-sharded Wq/Wkv),
RoPE, causal flash-style attention in transposed layout (S^T = K Q^T so the
PV contraction lands on partitions), then AllGathers the per-group attention
outputs O^T across the 4 cores of its batch and applies a column shard of Wo
(full contraction, no all-reduce needed).  Output per core: y[b][:, 512g:512(g+1)].

Scheduling: the PE queue is kept dense by interleaving attention head-pairs
per k-chunk (the other head's score matmul hides the exp latency) and by
draining "fill" work (next block's QKV projections, previous block's Wo
contraction) between attention chunks via generators.  This keeps the tensor
engine p-state at max and hides ACT/DVE latency.

All matmul operands are bf16 (1 cycle/row on PE); accumulation is f32 in PSUM;
softmax runs without max-subtraction (scores are ~N(0,1), exp is safe in f32).
Causal structure is exploited at column granularity: for a diagonal k-chunk at
offset d, only score columns >= d are computed/exp'd/accumulated, and the
staircase boundary is handled by one [128,128] triangle multiply adding -40
on masked slots (exp then yields ~0 with no vector-engine mask op).
"""

import math
import sys

sys.path.insert(0, "/opt/trn_rl_repo")

import numpy as np
import ml_dtypes

import concourse.bass as bass
import concourse.mybir as mybir
import concourse.tile as tile
from concourse import bacc
from concourse import bass_utils
from concourse.bass import ds, ts

BF16 = mybir.dt.bfloat16
F32 = mybir.dt.float32
AF = mybir.ActivationFunctionType

HD = 128      # head dim
GQ = 4        # query heads per core
QC = GQ * HD  # query columns per core (512)
SB = 512      # sequence block
P = 128


def build_kernel(S=2048, H=2048):
    NB = S // SB          # number of seq blocks
    HO = H // P           # hidden contraction chunks
    ST = SB // P          # seq tiles per block (4)
    OC = H // 4           # output columns per core
    NPAIR = 2             # head pairs per core (AG granularity)

    nc = bacc.Bacc("TRN2", target_bir_lowering=False, debug=False, num_devices=8)

    # all inputs host-pre-tiled so every DMA reads a contiguous DRAM block
    xt = nc.dram_tensor("xt", [NB, HO, P, SB], BF16, kind="ExternalInput").ap()
    wq = nc.dram_tensor("wq", [GQ, HO, P, HD], BF16, kind="ExternalInput").ap()
    wk = nc.dram_tensor("wk", [HO, P, HD], BF16, kind="ExternalInput").ap()
    wv = nc.dram_tensor("wv", [HO, P, HD], BF16, kind="ExternalInput").ap()
    wo = nc.dram_tensor("wo", [16, P, OC], BF16, kind="ExternalInput").ap()
    cost = nc.dram_tensor("cost", [NB, P, SB], F32, kind="ExternalInput").ap()
    sint = nc.dram_tensor("sint", [NB, P, SB], F32, kind="ExternalInput").ap()
    rotm = nc.dram_tensor("rotm", [HD, HD], BF16, kind="ExternalInput").ap()
    ident = nc.dram_tensor("ident", [HD, HD], BF16, kind="ExternalInput").ap()
    trineg = nc.dram_tensor("trineg", [HD, HD], BF16, kind="ExternalInput").ap()
    out = nc.dram_tensor("out", [S, OC], F32, kind="ExternalOutput").ap()

    with tile.TileContext(nc) as tc:
        with (
            tc.tile_pool(name="consts", bufs=1) as consts,
            tc.tile_pool(name="wpool", bufs=1) as wpool,
            tc.tile_pool(name="xtp", bufs=2) as xtp,
            tc.tile_pool(name="kvp", bufs=1) as kvp,
            tc.tile_pool(name="qfp", bufs=2) as qfp,
            tc.tile_pool(name="work", bufs=3) as work,
            tc.tile_pool(name="ptp", bufs=4) as ptp,
            tc.tile_pool(name="gp", bufs=2) as gp,
            tc.tile_pool(name="psQ", bufs=2, space="PSUM") as psQ,
            tc.tile_pool(name="psS", bufs=3, space="PSUM") as psS,
            tc.tile_pool(name="psO", bufs=2, space="PSUM") as psO,
            tc.tile_pool(name="psY", bufs=1, space="PSUM") as psY,
            tc.tile_pool(name="dram", bufs=1, space="DRAM") as dpool,
        ):
            # ---- critical-path prologue: K-projection inputs first so the
            # PE starts within a few us; per-head Wq tiles so attention can
            # begin after just heads 0/1 arrive; everything else deferred ----
            xt_tiles = {}
            xt0 = xtp.tile([P, HO, SB], BF16, name="xt_sb")
            wk_sb = wpool.tile([P, HO, HD], BF16, name="wk_sb")
            wv_sb = wpool.tile([P, HO, HD], BF16, name="wv_sb")
            for ho in range(HO):
                nc.sync.dma_start(wk_sb[:, ho, :], wk[ho])
                nc.sync.dma_start(xt0[:, ho, :], xt[0, ho])
            xt_tiles[0] = xt0
            for ho in range(HO):
                nc.sync.dma_start(wv_sb[:, ho, :], wv[ho])
            rotm_sb = consts.tile([P, HD], BF16, name="rotm_sb")
            nc.sync.dma_start(rotm_sb[:], rotm[:])
            cos_tiles, sin_tiles = [], []
            for j in range(NB):
                cos_tiles.append(consts.tile([P, SB], F32, name=f"cos_{j}"))
                sin_tiles.append(consts.tile([P, SB], F32, name=f"sin_{j}"))
            nc.sync.dma_start(cos_tiles[0][:], cost[0])
            nc.sync.dma_start(sin_tiles[0][:], sint[0])
            ident_sb = consts.tile([P, HD], BF16, name="ident_sb")
            nc.sync.dma_start(ident_sb[:], ident[:])
            trineg_sb = consts.tile([P, HD], BF16, name="trineg_sb")
            nc.sync.dma_start(trineg_sb[:], trineg[:])
            ones_sb = consts.tile([P, 1], BF16, name="ones_sb")
            nc.vector.memset(ones_sb[:], 1.0)
            wqh = []
            for qc in range(GQ):
                wqh.append(wpool.tile([P, HO, HD], BF16, name=f"wqh{qc}"))
            for qc in range(GQ):
                for ho in range(HO):
                    nc.sync.dma_start(wqh[qc][:, ho, :], wq[qc, ho])
                if qc == 1:
                    break
            # (wqh2/3, xt1, cos/sin 1-3, wo are emitted after qkv(0) below)

            # K^T and V for the whole sequence (grow per block)
            kT_sb = kvp.tile([P, S], BF16, name="kT_sb")   # [hd, s]
            v_sb = kvp.tile([P, S], BF16, name="v_sb")     # [s%128, kc*128+hd]

            ag_ins = [[None] * NPAIR for _ in range(NB)]
            ag_outs = [[None] * NPAIR for _ in range(NB)]
            for j in range(NB):
                for pr in range(NPAIR):
                    ag_ins[j][pr] = dpool.tile(
                        [2 * P, SB], BF16, name=f"ag_in_{j}_{pr}")
                    ag_outs[j][pr] = dpool.tile(
                        [8 * P, SB], BF16, name=f"ag_out_{j}_{pr}")

            # ---- split rope: the PSUM->SBUF copy (DVE) is emitted with the
            # projection; the rotation matmul + muls run >=1 fill step later
            # so the PE never waits on the copy ----
            rope_pend = []

            def flush_rope():
                while rope_pend:
                    rope_pend.pop(0)()

            def sched_rope(out_ap, ps_raw, j):
                q_raw = work.tile([P, SB], BF16, tag="qraw", name="q_raw")
                nc.vector.tensor_copy(q_raw[:], ps_raw[:])

                def rope_b():
                    ps_rot = psS.tile([P, SB], F32, tag="pss", name="ps_rot")
                    nc.tensor.matmul(ps_rot[:], rotm_sb[:], q_raw[:],
                                     start=True, stop=True)
                    t1 = work.tile([P, SB], F32, tag="t1", name="t1", bufs=2)
                    nc.vector.tensor_mul(t1[:], ps_raw[:], cos_tiles[j][:])
                    t2 = work.tile([P, SB], F32, tag="t2", name="t2", bufs=2)
                    nc.vector.tensor_mul(t2[:], ps_rot[:], sin_tiles[j][:])
                    nc.vector.tensor_add(out_ap, t1[:], t2[:])
                rope_pend.append(rope_b)

            q_map = {}

            def kv_steps(j):
                """Generator: K/V projection for block j in ~0.9us PE steps."""
                xt_sb = xt_tiles[j]
                ps_k = psQ.tile([P, SB], F32, tag="ps", name="ps_k")
                for g4 in range(4):
                    for ho in range(4 * g4, 4 * g4 + 4):
                        nc.tensor.matmul(
                            ps_k[:], wk_sb[:, ho, :], xt_sb[:, ho, :],
                            start=(ho == 0), stop=(ho == HO - 1),
                        )
                    flush_rope()
                    yield
                sched_rope(kT_sb[:, ts(j, SB)], ps_k, j)
                yield
                ps_v = psQ.tile([P, SB], F32, tag="ps", name="ps_v")
                for st in range(ST):
                    for ho in range(HO):
                        nc.tensor.matmul(
                            ps_v[:, ts(st, P)], xt_sb[:, ho, ts(st, P)], wv_sb[:, ho, :],
                            start=(ho == 0), stop=(ho == HO - 1),
                        )
                    if st == 0:
                        flush_rope()
                    yield
                nc.vector.tensor_copy(v_sb[:, ts(j, SB)], ps_v[:])
                yield

            def q_steps(j, qcs):
                """Generator: Q-head projections + rope for block j."""
                xt_sb = xt_tiles[j]
                if j not in q_map:
                    q_map[j] = qfp.tile([P, GQ, SB], BF16, name="q_all")
                q_all = q_map[j]
                for qc in qcs:
                    ps_q = psQ.tile([P, SB], F32, tag="ps", name="ps_q")
                    for g4 in range(4):
                        for ho in range(4 * g4, 4 * g4 + 4):
                            nc.tensor.matmul(
                                ps_q[:], wqh[qc][:, ho, :], xt_sb[:, ho, :],
                                start=(ho == 0), stop=(ho == HO - 1),
                            )
                        if g4 == 1:
                            flush_rope()
                        yield
                    sched_rope(q_all[:, qc, :], ps_q, j)
                    yield
                yield
                flush_rope()

            g_loaded = {}

            def g_load(j, pr):
                """Load the gathered O^T chunks of AG(j, pr) into SBUF."""
                ag_r = ag_outs[j][pr].rearrange("(c p) s -> p c s", p=P)
                g_cs = []
                for c in range(8):
                    g_c = gp.tile([P, SB], BF16, tag=f"g{c}", name=f"g_c{c}")
                    nc.sync.dma_start(g_c[:], ag_r[:, c, :])
                    g_cs.append(g_c)
                g_loaded[(j, pr)] = g_cs

            y_parts = {}

            def wo_steps(j, pr):
                """Generator: Wo contraction for (block j, pair pr)."""
                g_cs = g_loaded.pop((j, pr))
                for st in range(ST):
                    ps_y = psY.tile([P, OC], F32, tag="psy", name="ps_y")
                    for c in range(8):
                        r, q = c // 2, c % 2
                        ocg = 4 * r + 2 * pr + q
                        nc.tensor.matmul(
                            ps_y[:], g_cs[c][:, ts(st, P)], wo_sb[:, ocg, :],
                            start=(c == 0), stop=(c == 7),
                        )
                        if c == 3:
                            yield
                    if pr == 0:
                        yp = work.tile([P, OC], F32, tag="ypart",
                                       name="y_part", bufs=5)
                        nc.vector.tensor_copy(yp[:], ps_y[:])
                        y_parts[(j, st)] = yp
                    else:
                        y_sb = work.tile([P, OC], F32, tag="ysb", name="y_sb",
                                         bufs=2)
                        nc.vector.tensor_add(
                            y_sb[:], y_parts.pop((j, st))[:], ps_y[:])
                        nc.scalar.dma_start(
                            out[ds(j * SB + st * P, P), :], y_sb[:])
                    yield

            # ---- fill machinery ----
            fill_q = []

            def emit_fill(n):
                done = 0
                while fill_q and done < n:
                    try:
                        next(fill_q[0])
                        done += 1
                    except StopIteration:
                        fill_q.pop(0)

            def drain_fill():
                while fill_q:
                    try:
                        next(fill_q[0])
                    except StopIteration:
                        fill_q.pop(0)

            def attn_sweep(j, pr):
                """Causal attention for heads (2pr, 2pr+1) of query block j,
                with fill work interleaved between k-chunks."""
                h0, h1 = 2 * pr, 2 * pr + 1
                q_all = q_map[j]
                KC = 4 * (j + 1)
                ps_o = {}
                accs = {}
                for h in (h0, h1):
                    ps_o[h] = psO.tile([P, SB], F32, tag="pso", name="ps_o")
                    accs[h] = [
                        work.tile([P, SB], BF16, tag=f"acc{h % 2}a",
                                  name="acc_a", bufs=2),
                        work.tile([P, SB], BF16, tag=f"acc{h % 2}b",
                                  name="acc_b", bufs=2),
                    ]
                pts = {}
                for kc in range(KC):
                    diag = kc >= 4 * j
                    d = P * (kc - 4 * j) if diag else 0
                    for h in (h0, h1):
                        ps_s = psS.tile([P, SB], F32, tag="pss", name="ps_s")
                        nc.tensor.matmul(
                            ps_s[:, d:], kT_sb[:, ts(kc, P)], q_all[:, h, d:],
                            start=True, stop=not diag,
                        )
                        if diag:
                            nc.tensor.matmul(
                                ps_s[:, d:d + P], ident_sb[:], trineg_sb[:],
                                start=False, stop=True,
                            )
                        pt = ptp.tile([P, SB], BF16, tag=f"pt{h % 2}", name="pt")
                        nc.scalar.activation(pt[:, d:], ps_s[:, d:], AF.Exp)
                        pts[h] = pt
                    emit_fill(1)
                    for h in (h0, h1):
                        pt = pts[h]
                        acc = accs[h][kc % 2]
                        if kc < 2:
                            if d > 0:
                                nc.vector.memset(acc[:, :d], 0.0)
                            nc.vector.tensor_copy(acc[:, d:], pt[:, d:])
                        else:
                            nc.vector.tensor_add(acc[:, d:], acc[:, d:], pt[:, d:])
                        nc.tensor.matmul(
                            ps_o[h][:, d:], v_sb[:, ts(kc, P)], pt[:, d:],
                            start=(kc == 0), stop=(kc == KC - 1),
                        )
                for h in (h0, h1):
                    ps_d = psS.tile([1, SB], F32, tag="pss", name="ps_d")
                    nc.tensor.matmul(ps_d[:], ones_sb[:], accs[h][0][:],
                                     start=True, stop=False)
                    nc.tensor.matmul(ps_d[:], ones_sb[:], accs[h][1][:],
                                     start=False, stop=True)
                    recip = work.tile([1, SB], F32, tag="recip", name="recip")
                    nc.vector.reciprocal_approx_fast(recip[:], ps_d[:])
                    rb = work.tile([P, SB], F32, tag="rb", name="rb", bufs=2)
                    nc.gpsimd.partition_broadcast(rb[:], recip[:], channels=P)
                    o_sb = work.tile([P, SB], BF16, tag="osb", name="o_sb")
                    nc.vector.tensor_mul(o_sb[:], ps_o[h][:], rb[:])
                    nc.scalar.dma_start(ag_ins[j][pr][ts(h % 2, P), :], o_sb[:])
                    emit_fill(1)
                nc.gpsimd.collective_compute(
                    "AllGather", mybir.AluOpType.bypass,
                    replica_groups=[[0, 1, 2, 3], [4, 5, 6, 7]],
                    ins=[ag_ins[j][pr][:].opt()],
                    outs=[ag_outs[j][pr][:].opt()],
                )

            # ---- emission schedule ----
            # qkv(0): K, V, heads 0/1 inline (DMA-gated startup); heads 2/3
            # become sweep fill so attention starts as soon as q1 is roped.
            for _ in kv_steps(0):
                pass
            for _ in q_steps(0, (0, 1)):
                pass
            q0_tail = q_steps(0, (2, 3))
            fill_q.append(q0_tail)

            # deferred prologue: needed from iter 0's fill onward
            for qc in (2, 3):
                for ho in range(HO):
                    nc.sync.dma_start(wqh[qc][:, ho, :], wq[qc, ho])
            xt1 = xtp.tile([P, HO, SB], BF16, name="xt_sb")
            for ho in range(HO):
                nc.sync.dma_start(xt1[:, ho, :], xt[1, ho])
            xt_tiles[1] = xt1
            for j in range(1, NB):
                nc.sync.dma_start(cos_tiles[j][:], cost[j])
                nc.sync.dma_start(sin_tiles[j][:], sint[j])
            wo_sb = wpool.tile([P, 16, OC], BF16, name="wo_sb")
            for oc in range(16):
                nc.sync.dma_start(wo_sb[:, oc, :], wo[oc])

            for j in range(NB):
                if j + 1 < NB:
                    fill_q.append(kv_steps(j + 1))
                    fill_q.append(q_steps(j + 1, (0, 1, 2, 3)))
                if j == NB - 1 and j >= 1:
                    # last iter has no qkv fill: use wo(j-1) inside sweeps
                    g_load(j - 1, 0)
                    fill_q.append(wo_steps(j - 1, 0))
                attn_sweep(j, 0)
                if j == 0:
                    # heads 2/3 of block 0 must be projected+roped before
                    # sweep(0,1) reads them
                    while q0_tail in fill_q:
                        try:
                            next(q0_tail)
                        except StopIteration:
                            fill_q.remove(q0_tail)
                if j >= 1 and j < NB - 1:
                    g_load(j - 1, 0)
                    g_load(j - 1, 1)
                if j == NB - 1 and j >= 1:
                    g_load(j - 1, 1)
                    fill_q.append(wo_steps(j - 1, 1))
                attn_sweep(j, 1)
                if j + 2 < NB:
                    xt_n = xtp.tile([P, HO, SB], BF16, name="xt_sb")
                    for ho in range(HO):
                        nc.sync.dma_start(xt_n[:, ho, :], xt[j + 2, ho])
                    xt_tiles[j + 2] = xt_n
                if j >= 1 and j < NB - 1:
                    fill_q.append(wo_steps(j - 1, 0))
                    fill_q.append(wo_steps(j - 1, 1))
                drain_fill()

            # tail: wo for the last block
            g_load(NB - 1, 0)
            for _ in wo_steps(NB - 1, 0):
                pass
            g_load(NB - 1, 1)
            for _ in wo_steps(NB - 1, 1):
                pass

    return nc


def make_in_maps(x, cos, sin, Wq, Wkv, Wo, S=2048, H=2048):
    bf = ml_dtypes.bfloat16
    scale = 1.0 / math.sqrt(HD)
    NKVH = Wkv.shape[1] // (2 * HD)  # 4
    OC = H // 4

    Prot = np.zeros((HD, HD), np.float32)
    Prot[np.arange(64), np.arange(64) + 64] = -1.0
    Prot[np.arange(64) + 64, np.arange(64)] = 1.0
    rotm = np.ascontiguousarray(Prot.T).astype(bf)

    kk = np.arange(P)[:, None]
    w = np.arange(HD)[None, :]
    trineg_np = np.where(w < kk, -40.0, 0.0).astype(np.float32).astype(bf)
    ident_np = np.eye(HD, dtype=np.float32).astype(bf)

    cost = np.ascontiguousarray(cos.T).astype(np.float32)
    sint = np.ascontiguousarray(sin.T).astype(np.float32)

    NB, HO, P_, SB_ = S // 512, H // 128, 128, 512

    def tile_xt(xtT):      # [H, S] -> [NB, HO, P, SB]
        return np.ascontiguousarray(
            xtT.reshape(HO, P_, NB, SB_).transpose(2, 0, 1, 3))

    def tile_wq(w):        # [H, QC] -> [GQ, HO, P, HD]
        return np.ascontiguousarray(
            w.reshape(HO, P_, 4, HD).transpose(2, 0, 1, 3))

    def tile_w1(w):        # [H, HD] -> [HO, P, HD]
        return np.ascontiguousarray(w.reshape(HO, P_, HD))

    def tile_wo(w):        # [2048, OC] -> [16, P, OC]
        return np.ascontiguousarray(w.reshape(16, P_, OC))

    def tile_cs(cT):       # [HD, S] -> [NB, P, SB]
        return np.ascontiguousarray(
            cT.reshape(P_, NB, SB_).transpose(1, 0, 2))

    in_maps = []
    for c in range(8):
        b, g = c // 4, c % 4
        in_maps.append({
            "xt": tile_xt(np.ascontiguousarray(np.asarray(x)[b].T).astype(bf)),
            "wq": tile_wq((np.asarray(Wq)[:, QC * g:QC * (g + 1)] * scale).astype(bf)),
            "wk": tile_w1(np.asarray(Wkv)[:, HD * g:HD * (g + 1)].astype(bf)),
            "wv": tile_w1(np.asarray(Wkv)[
                :, NKVH * HD + HD * g:NKVH * HD + HD * (g + 1)].astype(bf)),
            "wo": tile_wo(np.asarray(Wo)[:, OC * g:OC * (g + 1)].astype(bf)),
            "cost": tile_cs(cost), "sint": tile_cs(sint), "rotm": rotm,
            "ident": ident_np, "trineg": trineg_np,
        })
    return in_maps


_CACHE = {}


def _get_nc(S=2048, H=2048):
    key = (S, H)
    if key not in _CACHE:
        nc = build_kernel(S, H)
        nc.compile()
        _CACHE[key] = nc
    return _CACHE[key]


def run(x, cos, sin, Wq, Wkv, Wo, trace=False):
    S, H = 2048, 2048
    nc = _get_nc(S, H)
    in_maps = make_in_maps(x, cos, sin, Wq, Wkv, Wo, S, H)
    res = bass_utils.run_bass_kernel_spmd(
        nc, in_maps, core_ids=list(range(8)), trace=trace
    )
    OC = H // 4
    y = np.empty((2, S, H), np.float32)
    for c in range(8):
        b, g = c // 4, c % 4
        y[b][:, OC * g:OC * (g + 1)] = res.results[c]["out"]
    return y, res


def kernel(x, cos, sin, Wq, Wkv, Wo):
    y, _ = run(x, cos, sin, Wq, Wkv, Wo, trace=False)
    return y


# revision 8
# speedup vs baseline: 1.1711x; 1.1021x over previous
"""Distributed Trainium2 Bass kernel for GQA attention (B=2, S=2048, H=2048,
NH=16, NKV=4, HD=128) across 8 NeuronCores.

Sharding: core c -> (batch b = c//4, kv-group g = c%4).  Each core computes
Q/K/V projections for its 4 query heads + 1 kv head (column-sharded Wq/Wkv),
RoPE, causal flash-style attention in transposed layout (S^T = K Q^T so the
PV contraction lands on partitions), then AllGathers the per-group attention
outputs O^T across the 4 cores of its batch and applies a column shard of Wo
(full contraction, no all-reduce needed).  Output per core: y[b][:, 512g:512(g+1)].

Scheduling: the PE queue is kept dense by interleaving attention head-pairs
per k-chunk (the other head's score matmul hides the exp latency) and by
draining "fill" work (next block's QKV projections, previous block's Wo
contraction) between attention chunks via generators.  This keeps the tensor
engine p-state at max and hides ACT/DVE latency.

All matmul operands are bf16 (1 cycle/row on PE); accumulation is f32 in PSUM;
softmax runs without max-subtraction (scores are ~N(0,1), exp is safe in f32).
Causal structure is exploited at column granularity: for a diagonal k-chunk at
offset d, only score columns >= d are computed/exp'd/accumulated, and the
staircase boundary is handled by one [128,128] triangle multiply adding -40
on masked slots (exp then yields ~0 with no vector-engine mask op).
"""

import math
import sys

sys.path.insert(0, "/opt/trn_rl_repo")

import numpy as np
import ml_dtypes

import concourse.bass as bass
import concourse.mybir as mybir
import concourse.tile as tile
from concourse import bacc
from concourse import bass_utils
from concourse.bass import ds, ts

BF16 = mybir.dt.bfloat16
F32 = mybir.dt.float32
AF = mybir.ActivationFunctionType

HD = 128      # head dim
GQ = 4        # query heads per core
QC = GQ * HD  # query columns per core (512)
SB = 512      # sequence block
P = 128


def build_kernel(S=2048, H=2048):
    NB = S // SB          # number of seq blocks
    HO = H // P           # hidden contraction chunks
    ST = SB // P          # seq tiles per block (4)
    OC = H // 4           # output columns per core
    NPAIR = 2             # head pairs per core (AG granularity)

    nc = bacc.Bacc("TRN2", target_bir_lowering=False, debug=False, num_devices=8)

    # inputs host-pre-tiled so each DMA delivers one multi-KB contiguous
    # run per partition (DMA cost is per (partition, run) descriptor)
    xt = nc.dram_tensor("xt", [NB, 4, P, 4, SB], BF16, kind="ExternalInput").ap()
    wq = nc.dram_tensor("wq", [GQ, P, HO, HD], BF16, kind="ExternalInput").ap()
    wk = nc.dram_tensor("wk", [P, HO, HD], BF16, kind="ExternalInput").ap()
    wv = nc.dram_tensor("wv", [P, HO, HD], BF16, kind="ExternalInput").ap()
    wo = nc.dram_tensor("wo", [2, P, 8, OC], BF16, kind="ExternalInput").ap()
    cost = nc.dram_tensor("cost", [NB, P, SB], F32, kind="ExternalInput").ap()
    sint = nc.dram_tensor("sint", [NB, P, SB], F32, kind="ExternalInput").ap()
    rotm = nc.dram_tensor("rotm", [HD, HD], BF16, kind="ExternalInput").ap()
    ident = nc.dram_tensor("ident", [HD, HD], BF16, kind="ExternalInput").ap()
    trineg = nc.dram_tensor("trineg", [HD, HD], BF16, kind="ExternalInput").ap()
    out = nc.dram_tensor("out", [S, OC], F32, kind="ExternalOutput").ap()

    with tile.TileContext(nc) as tc:
        with (
            tc.tile_pool(name="consts", bufs=1) as consts,
            tc.tile_pool(name="wpool", bufs=1) as wpool,
            tc.tile_pool(name="xtp", bufs=2) as xtp,
            tc.tile_pool(name="kvp", bufs=1) as kvp,
            tc.tile_pool(name="qfp", bufs=2) as qfp,
            tc.tile_pool(name="work", bufs=3) as work,
            tc.tile_pool(name="ptp", bufs=4) as ptp,
            tc.tile_pool(name="gp", bufs=2) as gp,
            tc.tile_pool(name="psQ", bufs=2, space="PSUM") as psQ,
            tc.tile_pool(name="psS", bufs=3, space="PSUM") as psS,
            tc.tile_pool(name="psO", bufs=2, space="PSUM") as psO,
            tc.tile_pool(name="psY", bufs=1, space="PSUM") as psY,
            tc.tile_pool(name="dram", bufs=1, space="DRAM") as dpool,
        ):
            # ---- critical-path prologue: K-projection inputs first so the
            # PE starts within a few us; per-head Wq tiles so attention can
            # begin after just heads 0/1 arrive; everything else deferred ----
            xt_tiles = {}
            xt0 = xtp.tile([P, HO, SB], BF16, name="xt_sb")
            wk_sb = wpool.tile([P, HO, HD], BF16, name="wk_sb")
            wv_sb = wpool.tile([P, HO, HD], BF16, name="wv_sb")
            nc.sync.dma_start(wk_sb[:], wk[:])
            for hg in range(4):
                nc.sync.dma_start(xt0[:, ts(hg, 4), :], xt[0, hg])
            xt_tiles[0] = xt0
            nc.sync.dma_start(wv_sb[:], wv[:])
            rotm_sb = consts.tile([P, HD], BF16, name="rotm_sb")
            nc.sync.dma_start(rotm_sb[:], rotm[:])
            cos_tiles, sin_tiles = [], []
            for j in range(NB):
                cos_tiles.append(consts.tile([P, SB], F32, name=f"cos_{j}"))
                sin_tiles.append(consts.tile([P, SB], F32, name=f"sin_{j}"))
            nc.sync.dma_start(cos_tiles[0][:], cost[0])
            nc.sync.dma_start(sin_tiles[0][:], sint[0])
            ident_sb = consts.tile([P, HD], BF16, name="ident_sb")
            nc.sync.dma_start(ident_sb[:], ident[:])
            trineg_sb = consts.tile([P, HD], BF16, name="trineg_sb")
            nc.sync.dma_start(trineg_sb[:], trineg[:])
            ones_sb = consts.tile([P, 1], BF16, name="ones_sb")
            nc.vector.memset(ones_sb[:], 1.0)
            wqh = []
            for qc in range(GQ):
                wqh.append(wpool.tile([P, HO, HD], BF16, name=f"wqh{qc}"))
            for qc in range(GQ):
                nc.sync.dma_start(wqh[qc][:], wq[qc])
                if qc == 1:
                    break
            # (wqh2/3, xt1, cos/sin 1-3, wo are emitted after qkv(0) below)

            # K^T and V for the whole sequence (grow per block)
            kT_sb = kvp.tile([P, S], BF16, name="kT_sb")   # [hd, s]
            v_sb = kvp.tile([P, S], BF16, name="v_sb")     # [s%128, kc*128+hd]

            ag_ins = [[None] * NPAIR for _ in range(NB)]
            ag_outs = [[None] * NPAIR for _ in range(NB)]
            for j in range(NB):
                for pr in range(NPAIR):
                    ag_ins[j][pr] = dpool.tile(
                        [2 * P, SB], BF16, name=f"ag_in_{j}_{pr}")
                    ag_outs[j][pr] = dpool.tile(
                        [8 * P, SB], BF16, name=f"ag_out_{j}_{pr}")

            # ---- split rope: the PSUM->SBUF copy (DVE) is emitted with the
            # projection; the rotation matmul + muls run >=1 fill step later
            # so the PE never waits on the copy ----
            rope_pend = []

            def flush_rope():
                while rope_pend:
                    rope_pend.pop(0)()

            def sched_rope(out_ap, ps_raw, j):
                q_raw = work.tile([P, SB], BF16, tag="qraw", name="q_raw")
                nc.vector.tensor_copy(q_raw[:], ps_raw[:])

                def rope_b():
                    ps_rot = psS.tile([P, SB], F32, tag="pss", name="ps_rot")
                    nc.tensor.matmul(ps_rot[:], rotm_sb[:], q_raw[:],
                                     start=True, stop=True)
                    t1 = work.tile([P, SB], F32, tag="t1", name="t1", bufs=2)
                    nc.vector.tensor_mul(t1[:], ps_raw[:], cos_tiles[j][:])
                    t2 = work.tile([P, SB], F32, tag="t2", name="t2", bufs=2)
                    nc.vector.tensor_mul(t2[:], ps_rot[:], sin_tiles[j][:])
                    nc.vector.tensor_add(out_ap, t1[:], t2[:])
                rope_pend.append(rope_b)

            q_map = {}

            def kv_steps(j):
                """Generator: K/V projection for block j in ~0.9us PE steps."""
                xt_sb = xt_tiles[j]
                ps_k = psQ.tile([P, SB], F32, tag="ps", name="ps_k")
                for g4 in range(4):
                    for ho in range(4 * g4, 4 * g4 + 4):
                        nc.tensor.matmul(
                            ps_k[:], wk_sb[:, ho, :], xt_sb[:, ho, :],
                            start=(ho == 0), stop=(ho == HO - 1),
                        )
                    flush_rope()
                    yield
                sched_rope(kT_sb[:, ts(j, SB)], ps_k, j)
                yield
                ps_v = psQ.tile([P, SB], F32, tag="ps", name="ps_v")
                for st in range(ST):
                    for ho in range(HO):
                        nc.tensor.matmul(
                            ps_v[:, ts(st, P)], xt_sb[:, ho, ts(st, P)], wv_sb[:, ho, :],
                            start=(ho == 0), stop=(ho == HO - 1),
                        )
                    if st == 0:
                        flush_rope()
                    yield
                nc.vector.tensor_copy(v_sb[:, ts(j, SB)], ps_v[:])
                yield

            def q_steps(j, qcs):
                """Generator: Q-head projections + rope for block j."""
                xt_sb = xt_tiles[j]
                if j not in q_map:
                    q_map[j] = qfp.tile([P, GQ, SB], BF16, name="q_all")
                q_all = q_map[j]
                for qc in qcs:
                    ps_q = psQ.tile([P, SB], F32, tag="ps", name="ps_q")
                    for g4 in range(4):
                        for ho in range(4 * g4, 4 * g4 + 4):
                            nc.tensor.matmul(
                                ps_q[:], wqh[qc][:, ho, :], xt_sb[:, ho, :],
                                start=(ho == 0), stop=(ho == HO - 1),
                            )
                        if g4 == 1:
                            flush_rope()
                        yield
                    sched_rope(q_all[:, qc, :], ps_q, j)
                    yield
                yield
                flush_rope()

            g_loaded = {}

            def g_load(j, pr):
                """Load the gathered O^T chunks of AG(j, pr) into SBUF."""
                ag_r = ag_outs[j][pr].rearrange("(c p) s -> p c s", p=P)
                g_cs = []
                for c in range(8):
                    g_c = gp.tile([P, SB], BF16, tag=f"g{c}", name=f"g_c{c}")
                    nc.sync.dma_start(g_c[:], ag_r[:, c, :])
                    g_cs.append(g_c)
                g_loaded[(j, pr)] = g_cs

            y_parts = {}

            def wo_steps(j, pr):
                """Generator: Wo contraction for (block j, pair pr)."""
                g_cs = g_loaded.pop((j, pr))
                for st in range(ST):
                    ps_y = psY.tile([P, OC], F32, tag="psy", name="ps_y")
                    for c in range(8):
                        r, q = c // 2, c % 2
                        ocg = 4 * r + 2 * pr + q
                        nc.tensor.matmul(
                            ps_y[:], g_cs[c][:, ts(st, P)], wo_sb[:, ocg, :],
                            start=(c == 0), stop=(c == 7),
                        )
                        if c == 3:
                            yield
                    if pr == 0:
                        yp = work.tile([P, OC], F32, tag="ypart",
                                       name="y_part", bufs=5)
                        nc.vector.tensor_copy(yp[:], ps_y[:])
                        y_parts[(j, st)] = yp
                    else:
                        y_sb = work.tile([P, OC], F32, tag="ysb", name="y_sb",
                                         bufs=2)
                        nc.vector.tensor_add(
                            y_sb[:], y_parts.pop((j, st))[:], ps_y[:])
                        nc.scalar.dma_start(
                            out[ds(j * SB + st * P, P), :], y_sb[:])
                    yield

            # ---- fill machinery ----
            fill_q = []

            def emit_fill(n):
                done = 0
                while fill_q and done < n:
                    try:
                        next(fill_q[0])
                        done += 1
                    except StopIteration:
                        fill_q.pop(0)

            def drain_fill():
                while fill_q:
                    try:
                        next(fill_q[0])
                    except StopIteration:
                        fill_q.pop(0)

            def attn_sweep(j, pr):
                """Causal attention for heads (2pr, 2pr+1) of query block j,
                with fill work interleaved between k-chunks."""
                h0, h1 = 2 * pr, 2 * pr + 1
                q_all = q_map[j]
                KC = 4 * (j + 1)
                ps_o = {}
                accs = {}
                for h in (h0, h1):
                    ps_o[h] = psO.tile([P, SB], F32, tag="pso", name="ps_o")
                    accs[h] = [
                        work.tile([P, SB], BF16, tag=f"acc{h % 2}a",
                                  name="acc_a", bufs=2),
                        work.tile([P, SB], BF16, tag=f"acc{h % 2}b",
                                  name="acc_b", bufs=2),
                    ]
                pts = {}
                for kc in range(KC):
                    diag = kc >= 4 * j
                    d = P * (kc - 4 * j) if diag else 0
                    for h in (h0, h1):
                        ps_s = psS.tile([P, SB], F32, tag="pss", name="ps_s")
                        nc.tensor.matmul(
                            ps_s[:, d:], kT_sb[:, ts(kc, P)], q_all[:, h, d:],
                            start=True, stop=not diag,
                        )
                        if diag:
                            nc.tensor.matmul(
                                ps_s[:, d:d + P], ident_sb[:], trineg_sb[:],
                                start=False, stop=True,
                            )
                        pt = ptp.tile([P, SB], BF16, tag=f"pt{h % 2}", name="pt")
                        nc.scalar.activation(pt[:, d:], ps_s[:, d:], AF.Exp)
                        pts[h] = pt
                    emit_fill(1)
                    for h in (h0, h1):
                        pt = pts[h]
                        acc = accs[h][kc % 2]
                        if kc < 2:
                            if d > 0:
                                nc.vector.memset(acc[:, :d], 0.0)
                            nc.vector.tensor_copy(acc[:, d:], pt[:, d:])
                        else:
                            nc.vector.tensor_add(acc[:, d:], acc[:, d:], pt[:, d:])
                        nc.tensor.matmul(
                            ps_o[h][:, d:], v_sb[:, ts(kc, P)], pt[:, d:],
                            start=(kc == 0), stop=(kc == KC - 1),
                        )
                for h in (h0, h1):
                    ps_d = psS.tile([1, SB], F32, tag="pss", name="ps_d")
                    nc.tensor.matmul(ps_d[:], ones_sb[:], accs[h][0][:],
                                     start=True, stop=False)
                    nc.tensor.matmul(ps_d[:], ones_sb[:], accs[h][1][:],
                                     start=False, stop=True)
                    recip = work.tile([1, SB], F32, tag="recip", name="recip")
                    nc.vector.reciprocal_approx_fast(recip[:], ps_d[:])
                    rb = work.tile([P, SB], F32, tag="rb", name="rb", bufs=2)
                    nc.gpsimd.partition_broadcast(rb[:], recip[:], channels=P)
                    o_sb = work.tile([P, SB], BF16, tag="osb", name="o_sb")
                    nc.vector.tensor_mul(o_sb[:], ps_o[h][:], rb[:])
                    nc.scalar.dma_start(ag_ins[j][pr][ts(h % 2, P), :], o_sb[:])
                    emit_fill(1)
                nc.gpsimd.collective_compute(
                    "AllGather", mybir.AluOpType.bypass,
                    replica_groups=[[0, 1, 2, 3], [4, 5, 6, 7]],
                    ins=[ag_ins[j][pr][:].opt()],
                    outs=[ag_outs[j][pr][:].opt()],
                )

            # ---- emission schedule ----
            # qkv(0): K, V, heads 0/1 inline (DMA-gated startup); heads 2/3
            # become sweep fill so attention starts as soon as q1 is roped.
            for _ in kv_steps(0):
                pass
            for _ in q_steps(0, (0, 1)):
                pass
            q0_tail = q_steps(0, (2, 3))
            fill_q.append(q0_tail)

            # deferred prologue: needed from iter 0's fill onward
            for qc in (2, 3):
                nc.sync.dma_start(wqh[qc][:], wq[qc])
            xt1 = xtp.tile([P, HO, SB], BF16, name="xt_sb")
            for hg in range(4):
                nc.sync.dma_start(xt1[:, ts(hg, 4), :], xt[1, hg])
            xt_tiles[1] = xt1
            for j in range(1, NB):
                nc.sync.dma_start(cos_tiles[j][:], cost[j])
                nc.sync.dma_start(sin_tiles[j][:], sint[j])
            wo_sb = wpool.tile([P, 16, OC], BF16, name="wo_sb")
            for half in range(2):
                nc.sync.dma_start(wo_sb[:, ts(half, 8), :], wo[half])

            for j in range(NB):
                if j + 1 < NB:
                    fill_q.append(kv_steps(j + 1))
                    fill_q.append(q_steps(j + 1, (0, 1, 2, 3)))
                if j == NB - 1 and j >= 1:
                    # last iter has no qkv fill: use wo(j-1) inside sweeps
                    g_load(j - 1, 0)
                    fill_q.append(wo_steps(j - 1, 0))
                attn_sweep(j, 0)
                if j == 0:
                    # heads 2/3 of block 0 must be projected+roped before
                    # sweep(0,1) reads them
                    while q0_tail in fill_q:
                        try:
                            next(q0_tail)
                        except StopIteration:
                            fill_q.remove(q0_tail)
                if j >= 1 and j < NB - 1:
                    g_load(j - 1, 0)
                    g_load(j - 1, 1)
                if j == NB - 1 and j >= 1:
                    g_load(j - 1, 1)
                    fill_q.append(wo_steps(j - 1, 1))
                attn_sweep(j, 1)
                if j + 2 < NB:
                    xt_n = xtp.tile([P, HO, SB], BF16, name="xt_sb")
                    for hg in range(4):
                        nc.sync.dma_start(xt_n[:, ts(hg, 4), :], xt[j + 2, hg])
                    xt_tiles[j + 2] = xt_n
                if j >= 1 and j < NB - 1:
                    fill_q.append(wo_steps(j - 1, 0))
                    fill_q.append(wo_steps(j - 1, 1))
                drain_fill()

            # tail: wo for the last block
            g_load(NB - 1, 0)
            for _ in wo_steps(NB - 1, 0):
                pass
            g_load(NB - 1, 1)
            for _ in wo_steps(NB - 1, 1):
                pass

    return nc


def make_in_maps(x, cos, sin, Wq, Wkv, Wo, S=2048, H=2048):
    bf = ml_dtypes.bfloat16
    scale = 1.0 / math.sqrt(HD)
    NKVH = Wkv.shape[1] // (2 * HD)  # 4
    OC = H // 4

    Prot = np.zeros((HD, HD), np.float32)
    Prot[np.arange(64), np.arange(64) + 64] = -1.0
    Prot[np.arange(64) + 64, np.arange(64)] = 1.0
    rotm = np.ascontiguousarray(Prot.T).astype(bf)

    kk = np.arange(P)[:, None]
    w = np.arange(HD)[None, :]
    trineg_np = np.where(w < kk, -40.0, 0.0).astype(np.float32).astype(bf)
    ident_np = np.eye(HD, dtype=np.float32).astype(bf)

    cost = np.ascontiguousarray(cos.T).astype(np.float32)
    sint = np.ascontiguousarray(sin.T).astype(np.float32)

    NB, HO, P_, SB_ = S // 512, H // 128, 128, 512

    def tile_xt(xtT):      # [H, S] -> [NB, 4, P, 4*SB]
        t = xtT.reshape(4, 4, P_, NB, SB_).transpose(3, 0, 2, 1, 4)
        return np.ascontiguousarray(t)

    def tile_wq(w):        # [H, QC] -> [GQ, P, HO*HD]
        t = w.reshape(HO, P_, 4, HD).transpose(2, 1, 0, 3)
        return np.ascontiguousarray(t)

    def tile_w1(w):        # [H, HD] -> [P, HO*HD]
        t = w.reshape(HO, P_, HD).transpose(1, 0, 2)
        return np.ascontiguousarray(t)

    def tile_wo(w):        # [2048, OC] -> [2, P, 8*OC]
        t = w.reshape(2, 8, P_, OC).transpose(0, 2, 1, 3)
        return np.ascontiguousarray(t)

    def tile_cs(cT):       # [HD, S] -> [NB, P, SB]
        return np.ascontiguousarray(
            cT.reshape(P_, NB, SB_).transpose(1, 0, 2))

    in_maps = []
    for c in range(8):
        b, g = c // 4, c % 4
        in_maps.append({
            "xt": tile_xt(np.ascontiguousarray(np.asarray(x)[b].T).astype(bf)),
            "wq": tile_wq((np.asarray(Wq)[:, QC * g:QC * (g + 1)] * scale).astype(bf)),
            "wk": tile_w1(np.asarray(Wkv)[:, HD * g:HD * (g + 1)].astype(bf)),
            "wv": tile_w1(np.asarray(Wkv)[
                :, NKVH * HD + HD * g:NKVH * HD + HD * (g + 1)].astype(bf)),
            "wo": tile_wo(np.asarray(Wo)[:, OC * g:OC * (g + 1)].astype(bf)),
            "cost": tile_cs(cost), "sint": tile_cs(sint), "rotm": rotm,
            "ident": ident_np, "trineg": trineg_np,
        })
    return in_maps


_CACHE = {}


def _get_nc(S=2048, H=2048):
    key = (S, H)
    if key not in _CACHE:
        nc = build_kernel(S, H)
        nc.compile()
        _CACHE[key] = nc
    return _CACHE[key]


def run(x, cos, sin, Wq, Wkv, Wo, trace=False):
    S, H = 2048, 2048
    nc = _get_nc(S, H)
    in_maps = make_in_maps(x, cos, sin, Wq, Wkv, Wo, S, H)
    res = bass_utils.run_bass_kernel_spmd(
        nc, in_maps, core_ids=list(range(8)), trace=trace
    )
    OC = H // 4
    y = np.empty((2, S, H), np.float32)
    for c in range(8):
        b, g = c // 4, c % 4
        y[b][:, OC * g:OC * (g + 1)] = res.results[c]["out"]
    return y, res


def kernel(x, cos, sin, Wq, Wkv, Wo):
    y, _ = run(x, cos, sin, Wq, Wkv, Wo, trace=False)
    return y


# revision 10
# speedup vs baseline: 1.2178x; 1.0399x over previous
"""Distributed Trainium2 Bass kernel for GQA attention (B=2, S=2048, H=2048,
NH=16, NKV=4, HD=128) across 8 NeuronCores.

Sharding: core c -> (batch b = c//4, kv-group g = c%4).  Each core computes
Q/K/V projections for its 4 query heads + 1 kv head (column-sharded Wq/Wkv),
RoPE, causal flash-style attention in transposed layout (S^T = K Q^T so the
PV contraction lands on partitions), then AllGathers the per-group attention
outputs O^T across the 4 cores of its batch and applies a column shard of Wo
(full contraction, no all-reduce needed).  Output per core: y[b][:, 512g:512(g+1)].

Scheduling: the PE queue is kept dense by interleaving attention head-pairs
per k-chunk (the other head's score matmul hides the exp latency) and by
draining "fill" work (next block's QKV projections, previous block's Wo
contraction) between attention chunks via generators.  This keeps the tensor
engine p-state at max and hides ACT/DVE latency.

All matmul operands are bf16 (1 cycle/row on PE); accumulation is f32 in PSUM;
softmax runs without max-subtraction (scores are ~N(0,1), exp is safe in f32).
Causal structure is exploited at column granularity: for a diagonal k-chunk at
offset d, only score columns >= d are computed/exp'd/accumulated, and the
staircase boundary is handled by one [128,128] triangle multiply adding -40
on masked slots (exp then yields ~0 with no vector-engine mask op).
"""

import math
import sys

sys.path.insert(0, "/opt/trn_rl_repo")

import numpy as np
import ml_dtypes

import concourse.bass as bass
import concourse.mybir as mybir
import concourse.tile as tile
from concourse import bacc
from concourse import bass_utils
from concourse.bass import ds, ts

BF16 = mybir.dt.bfloat16
F32 = mybir.dt.float32
AF = mybir.ActivationFunctionType

HD = 128      # head dim
GQ = 4        # query heads per core
QC = GQ * HD  # query columns per core (512)
SB = 512      # sequence block
P = 128


def build_kernel(S=2048, H=2048):
    NB = S // SB          # number of seq blocks
    HO = H // P           # hidden contraction chunks
    ST = SB // P          # seq tiles per block (4)
    OC = H // 4           # output columns per core
    NPAIR = 2             # head pairs per core (AG granularity)

    nc = bacc.Bacc("TRN2", target_bir_lowering=False, debug=False, num_devices=8)

    # inputs host-pre-tiled so each DMA delivers one multi-KB contiguous
    # run per partition (DMA cost is per (partition, run) descriptor)
    xt = nc.dram_tensor("xt", [NB, 4, P, 4, SB], BF16, kind="ExternalInput").ap()
    wq = nc.dram_tensor("wq", [GQ, P, HO, HD], BF16, kind="ExternalInput").ap()
    wk = nc.dram_tensor("wk", [P, HO, HD], BF16, kind="ExternalInput").ap()
    wv = nc.dram_tensor("wv", [P, HO, HD], BF16, kind="ExternalInput").ap()
    wo = nc.dram_tensor("wo", [2, P, 8, OC], BF16, kind="ExternalInput").ap()
    cs = nc.dram_tensor("cs", [NB, P, 2, SB], F32, kind="ExternalInput").ap()
    rotm = nc.dram_tensor("rotm", [HD, HD], BF16, kind="ExternalInput").ap()
    ident = nc.dram_tensor("ident", [HD, HD], BF16, kind="ExternalInput").ap()
    trineg = nc.dram_tensor("trineg", [HD, HD], BF16, kind="ExternalInput").ap()
    out = nc.dram_tensor("out", [S, OC], F32, kind="ExternalOutput").ap()

    with tile.TileContext(nc) as tc:
        with (
            tc.tile_pool(name="consts", bufs=1) as consts,
            tc.tile_pool(name="wpool", bufs=1) as wpool,
            tc.tile_pool(name="xtp", bufs=2) as xtp,
            tc.tile_pool(name="kvp", bufs=1) as kvp,
            tc.tile_pool(name="qfp", bufs=2) as qfp,
            tc.tile_pool(name="work", bufs=3) as work,
            tc.tile_pool(name="ptp", bufs=4) as ptp,
            tc.tile_pool(name="gp", bufs=2) as gp,
            tc.tile_pool(name="psQ", bufs=2, space="PSUM") as psQ,
            tc.tile_pool(name="psS", bufs=3, space="PSUM") as psS,
            tc.tile_pool(name="psO", bufs=2, space="PSUM") as psO,
            tc.tile_pool(name="psY", bufs=1, space="PSUM") as psY,
            tc.tile_pool(name="dram", bufs=1, space="DRAM") as dpool,
        ):
            # ---- critical-path prologue: K-projection inputs first so the
            # PE starts within a few us; per-head Wq tiles so attention can
            # begin after just heads 0/1 arrive; everything else deferred ----
            xt_tiles = {}
            xt0 = xtp.tile([P, HO, SB], BF16, name="xt_sb")
            wk_sb = wpool.tile([P, HO, HD], BF16, name="wk_sb")
            wv_sb = wpool.tile([P, HO, HD], BF16, name="wv_sb")
            nc.sync.dma_start(wk_sb[:], wk[:])
            for hg in range(4):
                nc.sync.dma_start(xt0[:, ts(hg, 4), :], xt[0, hg])
            xt_tiles[0] = xt0
            nc.sync.dma_start(wv_sb[:], wv[:])
            rotm_sb = consts.tile([P, HD], BF16, name="rotm_sb")
            nc.sync.dma_start(rotm_sb[:], rotm[:])
            cs_tiles = []
            for j in range(NB):
                cs_tiles.append(consts.tile([P, 2, SB], F32, name=f"cs_{j}"))
            cos_tiles = [t[:, 0, :] for t in cs_tiles]
            sin_tiles = [t[:, 1, :] for t in cs_tiles]
            nc.sync.dma_start(cs_tiles[0][:], cs[0])
            ident_sb = consts.tile([P, HD], BF16, name="ident_sb")
            nc.sync.dma_start(ident_sb[:], ident[:])
            trineg_sb = consts.tile([P, HD], BF16, name="trineg_sb")
            nc.sync.dma_start(trineg_sb[:], trineg[:])
            ones_sb = consts.tile([P, 1], BF16, name="ones_sb")
            nc.vector.memset(ones_sb[:], 1.0)
            wqh = []
            for qc in range(GQ):
                wqh.append(wpool.tile([P, HO, HD], BF16, name=f"wqh{qc}"))
            for qc in range(GQ):
                nc.sync.dma_start(wqh[qc][:], wq[qc])
                if qc == 1:
                    break
            # (wqh2/3, xt1, cos/sin 1-3, wo are emitted after qkv(0) below)

            # K^T and V for the whole sequence (grow per block)
            kT_sb = kvp.tile([P, S], BF16, name="kT_sb")   # [hd, s]
            v_sb = kvp.tile([P, S], BF16, name="v_sb")     # [s%128, kc*128+hd]

            ag_ins = [[None] * NPAIR for _ in range(NB)]
            ag_outs = [[None] * NPAIR for _ in range(NB)]
            for j in range(NB):
                for pr in range(NPAIR):
                    ag_ins[j][pr] = dpool.tile(
                        [2 * P, SB], BF16, name=f"ag_in_{j}_{pr}")
                    ag_outs[j][pr] = dpool.tile(
                        [8 * P, SB], BF16, name=f"ag_out_{j}_{pr}")

            # ---- split rope: the PSUM->SBUF copy (DVE) is emitted with the
            # projection; the rotation matmul + muls run >=1 fill step later
            # so the PE never waits on the copy ----
            rope_pend = []

            def flush_rope():
                while rope_pend:
                    rope_pend.pop(0)()

            def sched_rope(out_ap, ps_raw, j):
                q_raw = work.tile([P, SB], BF16, tag="qraw", name="q_raw")
                nc.vector.tensor_copy(q_raw[:], ps_raw[:])

                def rope_b():
                    ps_rot = psS.tile([P, SB], F32, tag="pss", name="ps_rot")
                    nc.tensor.matmul(ps_rot[:], rotm_sb[:], q_raw[:],
                                     start=True, stop=True)
                    t1 = work.tile([P, SB], F32, tag="t1", name="t1", bufs=2)
                    nc.vector.tensor_mul(t1[:], ps_raw[:], cos_tiles[j])
                    t2 = work.tile([P, SB], F32, tag="t2", name="t2", bufs=2)
                    nc.vector.tensor_mul(t2[:], ps_rot[:], sin_tiles[j])
                    nc.vector.tensor_add(out_ap, t1[:], t2[:])
                rope_pend.append(rope_b)

            q_map = {}

            def kv_steps(j):
                """Generator: K/V projection for block j in ~0.9us PE steps."""
                xt_sb = xt_tiles[j]
                ps_k = psQ.tile([P, SB], F32, tag="ps", name="ps_k")
                for g4 in range(4):
                    for ho in range(4 * g4, 4 * g4 + 4):
                        nc.tensor.matmul(
                            ps_k[:], wk_sb[:, ho, :], xt_sb[:, ho, :],
                            start=(ho == 0), stop=(ho == HO - 1),
                        )
                    flush_rope()
                    yield
                sched_rope(kT_sb[:, ts(j, SB)], ps_k, j)
                yield
                ps_v = psQ.tile([P, SB], F32, tag="ps", name="ps_v")
                for st in range(ST):
                    for ho in range(HO):
                        nc.tensor.matmul(
                            ps_v[:, ts(st, P)], xt_sb[:, ho, ts(st, P)], wv_sb[:, ho, :],
                            start=(ho == 0), stop=(ho == HO - 1),
                        )
                    if st == 0:
                        flush_rope()
                    yield
                nc.vector.tensor_copy(v_sb[:, ts(j, SB)], ps_v[:])
                yield

            def q_steps(j, qcs):
                """Generator: Q-head projections + rope for block j."""
                xt_sb = xt_tiles[j]
                if j not in q_map:
                    q_map[j] = qfp.tile([P, GQ, SB], BF16, name="q_all")
                q_all = q_map[j]
                for qc in qcs:
                    ps_q = psQ.tile([P, SB], F32, tag="ps", name="ps_q")
                    for g4 in range(4):
                        for ho in range(4 * g4, 4 * g4 + 4):
                            nc.tensor.matmul(
                                ps_q[:], wqh[qc][:, ho, :], xt_sb[:, ho, :],
                                start=(ho == 0), stop=(ho == HO - 1),
                            )
                        if g4 == 1:
                            flush_rope()
                        yield
                    sched_rope(q_all[:, qc, :], ps_q, j)
                    yield
                yield
                flush_rope()

            g_loaded = {}

            def g_load(j, pr):
                """Load the gathered O^T chunks of AG(j, pr) into SBUF."""
                ag_r = ag_outs[j][pr].rearrange("(c p) s -> p c s", p=P)
                g_cs = []
                for c in range(8):
                    g_c = gp.tile([P, SB], BF16, tag=f"g{c}", name=f"g_c{c}")
                    nc.sync.dma_start(g_c[:], ag_r[:, c, :])
                    g_cs.append(g_c)
                g_loaded[(j, pr)] = g_cs

            y_parts = {}

            def wo_steps(j, pr):
                """Generator: Wo contraction for (block j, pair pr)."""
                g_cs = g_loaded.pop((j, pr))
                for st in range(ST):
                    ps_y = psY.tile([P, OC], F32, tag="psy", name="ps_y")
                    for c in range(8):
                        r, q = c // 2, c % 2
                        ocg = 4 * r + 2 * pr + q
                        nc.tensor.matmul(
                            ps_y[:], g_cs[c][:, ts(st, P)], wo_sb[:, ocg, :],
                            start=(c == 0), stop=(c == 7),
                        )
                        if c == 3:
                            yield
                    if pr == 0:
                        yp = work.tile([P, OC], F32, tag="ypart",
                                       name="y_part", bufs=5)
                        nc.vector.tensor_copy(yp[:], ps_y[:])
                        y_parts[(j, st)] = yp
                    else:
                        y_sb = work.tile([P, OC], F32, tag="ysb", name="y_sb",
                                         bufs=2)
                        nc.vector.tensor_add(
                            y_sb[:], y_parts.pop((j, st))[:], ps_y[:])
                        nc.scalar.dma_start(
                            out[ds(j * SB + st * P, P), :], y_sb[:])
                    yield

            # ---- fill machinery ----
            fill_q = []

            def emit_fill(n):
                done = 0
                while fill_q and done < n:
                    try:
                        next(fill_q[0])
                        done += 1
                    except StopIteration:
                        fill_q.pop(0)

            def drain_fill():
                while fill_q:
                    try:
                        next(fill_q[0])
                    except StopIteration:
                        fill_q.pop(0)

            def attn_sweep(j, pr, mid_hook=None):
                """Causal attention for heads (2pr, 2pr+1) of query block j,
                with fill work interleaved between k-chunks."""
                h0, h1 = 2 * pr, 2 * pr + 1
                q_all = q_map[j]
                KC = 4 * (j + 1)
                ps_o = {}
                accs = {}
                for h in (h0, h1):
                    ps_o[h] = psO.tile([P, SB], F32, tag="pso", name="ps_o")
                    accs[h] = [
                        work.tile([P, SB], BF16, tag=f"acc{h % 2}a",
                                  name="acc_a", bufs=2),
                        work.tile([P, SB], BF16, tag=f"acc{h % 2}b",
                                  name="acc_b", bufs=2),
                    ]
                pts = {}
                for kc in range(KC):
                    diag = kc >= 4 * j
                    d = P * (kc - 4 * j) if diag else 0
                    for h in (h0, h1):
                        ps_s = psS.tile([P, SB], F32, tag="pss", name="ps_s")
                        nc.tensor.matmul(
                            ps_s[:, d:], kT_sb[:, ts(kc, P)], q_all[:, h, d:],
                            start=True, stop=not diag,
                        )
                        if diag:
                            nc.tensor.matmul(
                                ps_s[:, d:d + P], ident_sb[:], trineg_sb[:],
                                start=False, stop=True,
                            )
                        pt = ptp.tile([P, SB], BF16, tag=f"pt{h % 2}", name="pt")
                        nc.scalar.activation(pt[:, d:], ps_s[:, d:], AF.Exp)
                        pts[h] = pt
                    emit_fill(1)
                    if mid_hook is not None and kc == KC // 2:
                        mid_hook()
                        mid_hook = None
                    for h in (h0, h1):
                        pt = pts[h]
                        acc = accs[h][kc % 2]
                        if kc < 2:
                            if d > 0:
                                nc.vector.memset(acc[:, :d], 0.0)
                            nc.vector.tensor_copy(acc[:, d:], pt[:, d:])
                        else:
                            nc.vector.tensor_add(acc[:, d:], acc[:, d:], pt[:, d:])
                        nc.tensor.matmul(
                            ps_o[h][:, d:], v_sb[:, ts(kc, P)], pt[:, d:],
                            start=(kc == 0), stop=(kc == KC - 1),
                        )
                for h in (h0, h1):
                    ps_d = psS.tile([1, SB], F32, tag="pss", name="ps_d")
                    nc.tensor.matmul(ps_d[:], ones_sb[:], accs[h][0][:],
                                     start=True, stop=False)
                    nc.tensor.matmul(ps_d[:], ones_sb[:], accs[h][1][:],
                                     start=False, stop=True)
                    recip = work.tile([1, SB], F32, tag="recip", name="recip")
                    nc.vector.reciprocal_approx_fast(recip[:], ps_d[:])
                    rb = work.tile([P, SB], F32, tag="rb", name="rb", bufs=2)
                    nc.gpsimd.partition_broadcast(rb[:], recip[:], channels=P)
                    o_sb = work.tile([P, SB], BF16, tag="osb", name="o_sb")
                    nc.vector.tensor_mul(o_sb[:], ps_o[h][:], rb[:])
                    nc.scalar.dma_start(ag_ins[j][pr][ts(h % 2, P), :], o_sb[:])
                    emit_fill(1)
                nc.gpsimd.collective_compute(
                    "AllGather", mybir.AluOpType.bypass,
                    replica_groups=[[0, 1, 2, 3], [4, 5, 6, 7]],
                    ins=[ag_ins[j][pr][:].opt()],
                    outs=[ag_outs[j][pr][:].opt()],
                )

            # ---- emission schedule ----
            # qkv(0): K, V, heads 0/1 inline (DMA-gated startup); heads 2/3
            # become sweep fill so attention starts as soon as q1 is roped.
            for _ in kv_steps(0):
                pass
            for _ in q_steps(0, (0, 1)):
                pass
            q0_tail = q_steps(0, (2, 3))
            fill_q.append(q0_tail)

            # deferred prologue: needed from iter 0's fill onward
            for qc in (2, 3):
                nc.sync.dma_start(wqh[qc][:], wq[qc])
            xt1 = xtp.tile([P, HO, SB], BF16, name="xt_sb")
            for hg in range(4):
                nc.sync.dma_start(xt1[:, ts(hg, 4), :], xt[1, hg])
            xt_tiles[1] = xt1
            for j in range(1, NB):
                nc.sync.dma_start(cs_tiles[j][:], cs[j])
            wo_sb = wpool.tile([P, 16, OC], BF16, name="wo_sb")
            for half in range(2):
                nc.sync.dma_start(wo_sb[:, ts(half, 8), :], wo[half])

            # wo(j, 0) is consumed at the end of iter j+1; wo(j, 1) one
            # iteration later (AG(j,1) finishes late).  Block 3: wo(3,0)
            # overlaps sweep(3,1); only wo(3,1) remains after the last AG.
            for j in range(NB):
                if j + 1 < NB:
                    fill_q.append(kv_steps(j + 1))
                    fill_q.append(q_steps(j + 1, (0, 1, 2, 3)))
                if j == NB - 1:
                    g_load(j - 2, 1)
                    g_load(j - 1, 0)
                    fill_q.append(wo_steps(j - 2, 1))
                    fill_q.append(wo_steps(j - 1, 0))
                attn_sweep(j, 0)
                if j == 0:
                    # heads 2/3 of block 0 must be projected+roped before
                    # sweep(0,1) reads them
                    while q0_tail in fill_q:
                        try:
                            next(q0_tail)
                        except StopIteration:
                            fill_q.remove(q0_tail)
                if 1 <= j < NB - 1:
                    g_load(j - 1, 0)
                    if j >= 2:
                        g_load(j - 2, 1)
                if j == NB - 1:
                    g_load(j - 1, 1)
                    fill_q.append(wo_steps(j - 1, 1))
                    attn_sweep(j, 1, mid_hook=lambda: g_load(j, 0))
                else:
                    attn_sweep(j, 1)
                if j + 2 < NB:
                    xt_n = xtp.tile([P, HO, SB], BF16, name="xt_sb")
                    for hg in range(4):
                        nc.sync.dma_start(xt_n[:, ts(hg, 4), :], xt[j + 2, hg])
                    xt_tiles[j + 2] = xt_n
                if 1 <= j < NB - 1:
                    if j >= 2:
                        fill_q.append(wo_steps(j - 2, 1))
                    fill_q.append(wo_steps(j - 1, 0))
                if j == NB - 1:
                    fill_q.append(wo_steps(j, 0))
                drain_fill()

            # tail: only the last pair's wo remains
            g_load(NB - 1, 1)
            for _ in wo_steps(NB - 1, 1):
                pass

    return nc


def make_in_maps(x, cos, sin, Wq, Wkv, Wo, S=2048, H=2048):
    bf = ml_dtypes.bfloat16
    scale = 1.0 / math.sqrt(HD)
    NKVH = Wkv.shape[1] // (2 * HD)  # 4
    OC = H // 4

    Prot = np.zeros((HD, HD), np.float32)
    Prot[np.arange(64), np.arange(64) + 64] = -1.0
    Prot[np.arange(64) + 64, np.arange(64)] = 1.0
    rotm = np.ascontiguousarray(Prot.T).astype(bf)

    kk = np.arange(P)[:, None]
    w = np.arange(HD)[None, :]
    trineg_np = np.where(w < kk, -40.0, 0.0).astype(np.float32).astype(bf)
    ident_np = np.eye(HD, dtype=np.float32).astype(bf)

    cost = np.ascontiguousarray(cos.T).astype(np.float32)
    sint = np.ascontiguousarray(sin.T).astype(np.float32)

    NB, HO, P_, SB_ = S // 512, H // 128, 128, 512

    def tile_xt(xtT):      # [H, S] -> [NB, 4, P, 4*SB]
        t = xtT.reshape(4, 4, P_, NB, SB_).transpose(3, 0, 2, 1, 4)
        return np.ascontiguousarray(t)

    def tile_wq(w):        # [H, QC] -> [GQ, P, HO*HD]
        t = w.reshape(HO, P_, 4, HD).transpose(2, 1, 0, 3)
        return np.ascontiguousarray(t)

    def tile_w1(w):        # [H, HD] -> [P, HO*HD]
        t = w.reshape(HO, P_, HD).transpose(1, 0, 2)
        return np.ascontiguousarray(t)

    def tile_wo(w):        # [2048, OC] -> [2, P, 8*OC]
        t = w.reshape(2, 8, P_, OC).transpose(0, 2, 1, 3)
        return np.ascontiguousarray(t)

    def tile_cs(cT, sT):   # [HD, S] x2 -> [NB, P, 2, SB]
        c = cT.reshape(P_, NB, SB_).transpose(1, 0, 2)
        s = sT.reshape(P_, NB, SB_).transpose(1, 0, 2)
        return np.ascontiguousarray(np.stack([c, s], axis=2))

    in_maps = []
    for c in range(8):
        b, g = c // 4, c % 4
        in_maps.append({
            "xt": tile_xt(np.ascontiguousarray(np.asarray(x)[b].T).astype(bf)),
            "wq": tile_wq((np.asarray(Wq)[:, QC * g:QC * (g + 1)] * scale).astype(bf)),
            "wk": tile_w1(np.asarray(Wkv)[:, HD * g:HD * (g + 1)].astype(bf)),
            "wv": tile_w1(np.asarray(Wkv)[
                :, NKVH * HD + HD * g:NKVH * HD + HD * (g + 1)].astype(bf)),
            "wo": tile_wo(np.asarray(Wo)[:, OC * g:OC * (g + 1)].astype(bf)),
            "cs": tile_cs(cost, sint), "rotm": rotm,
            "ident": ident_np, "trineg": trineg_np,
        })
    return in_maps


_CACHE = {}


def _get_nc(S=2048, H=2048):
    key = (S, H)
    if key not in _CACHE:
        nc = build_kernel(S, H)
        nc.compile()
        _CACHE[key] = nc
    return _CACHE[key]


def run(x, cos, sin, Wq, Wkv, Wo, trace=False):
    S, H = 2048, 2048
    nc = _get_nc(S, H)
    in_maps = make_in_maps(x, cos, sin, Wq, Wkv, Wo, S, H)
    res = bass_utils.run_bass_kernel_spmd(
        nc, in_maps, core_ids=list(range(8)), trace=trace
    )
    OC = H // 4
    y = np.empty((2, S, H), np.float32)
    for c in range(8):
        b, g = c // 4, c % 4
        y[b][:, OC * g:OC * (g + 1)] = res.results[c]["out"]
    return y, res


def kernel(x, cos, sin, Wq, Wkv, Wo):
    y, _ = run(x, cos, sin, Wq, Wkv, Wo, trace=False)
    return y
